# revision 1
# baseline (speedup 1.0000x reference)
"""Trainium2 Bass kernel for a DeepseekV2 decoder-layer attention block
(MLA prefill, fp32 reference) distributed across 8 NeuronCores.

Strategy (single NEFF, SPMD on 8 cores):
  - Sequence-shard the shared projections: each core computes q_lora / ckv /
    k_pe (RMS-normed / roped) for its 256 rows of the sequence, in transposed
    layout, then two on-device AllGathers replicate them.
  - Head-shard the rest (4 heads per core): q_b projection + RoPE, kc/vc
    expansion, causal attention (scores computed transposed so the attn@v
    matmul needs no transposes), and a row-shard of w_o.
  - Each core emits a partial [S, HID] output; the host sums the 8 partials
    (the output all-reduce) to produce the full result.

All heavy matmuls run in bf16 with fp32 PSUM accumulation; softmax runs in
fp32 on the scalar engine (no max-subtraction needed: scores are O(3) by
construction and masked lanes underflow exp to exactly 0).

RMS-norm weights, the 1/sqrt(DQ) score scale, and the interleaved->half RoPE
permutation are folded into weights on the host. RoPE rotation is computed
via extra matmuls with sign-flipped, permuted weight columns so every
elementwise op stays partition-aligned.
"""

import numpy as np

S, HID, H = 2048, 5120, 32
QLR, KVLR = 1536, 512
DN, DR, DV = 128, 64, 128
DQ = DN + DR
NC_N = 8
HPC = H // NC_N          # heads per core
SL = S // NC_N           # sequence rows per core (front end)
ROPE_BASE, EPS = 10000.0, 1e-6

_CACHE = {}


def _bf16():
    import ml_dtypes
    return np.dtype(ml_dtypes.bfloat16)


def _build_program():
    import concourse.bass as bass
    import concourse.tile as tile
    from concourse import bacc, mybir
    from contextlib import ExitStack

    f32 = mybir.dt.float32
    bf = mybir.dt.bfloat16
    AF = mybir.ActivationFunctionType

    nc = bacc.Bacc("TRN2", target_bir_lowering=False, debug=False,
                   num_devices=NC_N)

    def din(name, shape, dt=bf):
        return nc.dram_tensor(name, list(shape), dt, kind="ExternalInput").ap()

    hsT_d = din("hsT", (HID, SL))
    wqa_d = din("wqa", (HID, QLR))
    wkvc_d = din("wkvc", (HID, KVLR))
    wkperot_d = din("wkperot", (HID, 256))
    cosl_d = din("cosl", (128, SL))
    sinl_d = din("sinl", (128, SL))
    cosf_d = din("cosf", (128, S))
    sinf_d = din("sinf", (128, S))
    wqb_d = din("wqb", (QLR, 1024))          # nope(4x128) | pe(2x128) | rot(2x128)
    kct_d = din("kct", (HPC * KVLR, DN))     # per head: kc'^T [KVLR, DN]
    vcp_d = din("vcp", (KVLR, HPC * DV))
    wo_d = din("wo", (HPC * DV, HID))
    masks_d = din("masks", (4 * 128, 512))   # additive causal masks (scores^T)
    out_d = nc.dram_tensor("out_partial", [S, HID], f32,
                           kind="ExternalOutput").ap()

    cc1_in = nc.dram_tensor("cc1_in", [KVLR + 128, SL], bf).ap()
    cc1_out = nc.dram_tensor("cc1_out", [NC_N * (KVLR + 128), SL], bf,
                             addr_space="Shared").ap()
    cc2_in = nc.dram_tensor("cc2_in", [QLR, SL], bf).ap()
    cc2a_out = nc.dram_tensor("cc2a_out", [NC_N * (QLR // 2), SL], bf,
                              addr_space="Shared").ap()
    cc2b_out = nc.dram_tensor("cc2b_out", [NC_N * (QLR // 2), SL], bf,
                              addr_space="Shared").ap()

    KH = HID // 128       # 40 k-chunks of the model dim
    KQ = QLR // 128       # 12 chunks of the q-lora dim
    KC = KVLR // 128      # 4 chunks of the kv-lora dim
    SC = S // 512         # 4 sequence chunks of 512
    SB = S // 128         # 16 sequence blocks of 128

    with tile.TileContext(nc) as tc, ExitStack() as ctx:
        def pool(name, bufs):
            return ctx.enter_context(tc.tile_pool(name=name, bufs=bufs))

        p_hs = pool("hs", 10)
        p_w = pool("wstr", 3)
        p_raw = pool("raw", 16)
        p_sq = pool("sqt", 2)
        p_scn = pool("scn", 3)
        p_f32 = pool("fr32", 2)
        p_sml = pool("sml", 8)
        p_one = pool("ones", 2)
        p_cs = pool("cs", 4)
        p_csl = pool("csl", 2)
        p_ckvg = pool("ckvg", 16)
        p_kpeg = pool("kpeg", 4)
        p_qlg = pool("qlg", 12)
        p_K = pool("Kt", 4)
        p_V = pool("Vt", 16)
        p_wqb = pool("wqb", 12)
        p_kc = pool("kc", 16)
        p_vc = pool("vc", 4)
        p_Qn = pool("Qn", 6)
        p_rope = pool("rope", 2)
        p_P = pool("Pt", 3)
        p_oT = pool("oT", 6)
        p_msk = pool("msk", 4)
        p_bc = pool("bc", 2)
        p_wo = pool("wo", 8)
        p_out = pool("outst", 2)

        pp_mm = ctx.enter_context(
            tc.tile_pool(name="pmm", bufs=7, space="PSUM"))
        pp_sm = ctx.enter_context(
            tc.tile_pool(name="psm", bufs=1, space="PSUM"))

        ones_col = p_one.tile([128, 1], bf)       # lhsT for column sums
        nc.vector.memset(ones_col[:], 1.0)
        eps_t = p_one.tile([1, 1], f32, tag="eps", name="eps")
        nc.vector.memset(eps_t[:], EPS)

        # ---------------- FRONT: sequence-sharded projections -------------
        # q_lora pass first (its AllGather is the big one and overlaps the
        # ckv pass + AG1 that follow). 6 accumulation chains per pass-group,
        # one PSUM bank each. hs/wqa stream on the sync HWDGE ring, wkv/wpr
        # on the scalar ring, cc writes + broadcasts + collectives on gpsimd.
        ssq_q = pp_sm.tile([1, SL], f32, tag="sm", name="sm")
        raw_q = []
        for g in range(2):
            ql_ps = [pp_mm.tile([128, SL], f32, tag="mm", name="mm")
                     for _ in range(KQ // 2)]
            for k in range(KH):
                hst = p_hs.tile([128, SL], bf, tag="hs", name="hs")
                nc.sync.dma_start(hst[:], hsT_d[k * 128:(k + 1) * 128, :])
                w = p_w.tile([128, QLR // 2], bf, tag="wqa", name="wqa", bufs=3)
                nc.sync.dma_start(
                    w[:], wqa_d[k * 128:(k + 1) * 128,
                                g * (QLR // 2):(g + 1) * (QLR // 2)])
                for mi in range(KQ // 2):
                    nc.tensor.matmul(ql_ps[mi][:],
                                     w[:, mi * 128:(mi + 1) * 128], hst[:],
                                     start=(k == 0), stop=(k == KH - 1))
            for mi in range(KQ // 2):
                m = g * (KQ // 2) + mi
                r = p_raw.tile([128, SL], bf, tag="raw", name="raw")
                nc.scalar.activation(r[:], ql_ps[mi][:], AF.Copy)
                raw_q.append(r)
                sq = p_sq.tile([128, SL], bf, tag="sq", name="sq")
                nc.scalar.activation(sq[:], ql_ps[mi][:], AF.Square)
                nc.tensor.matmul(ssq_q[:], ones_col[:], sq[:],
                                 start=(m == 0), stop=(m == KQ - 1))
        t_q = p_sml.tile([1, SL], f32, tag="sml", name="sml")
        nc.scalar.activation(t_q[:], ssq_q[:], AF.Sqrt,
                             bias=eps_t[:], scale=1.0 / QLR)
        s_q = p_sml.tile([1, SL], f32, tag="sml", name="sml")
        nc.vector.reciprocal(s_q[:], t_q[:])
        bq_sb = p_bc.tile([128, 512], f32, tag="bc", name="bc")
        nc.gpsimd.partition_broadcast(bq_sb[:, :SL], s_q[:])
        for m in range(KQ):
            qn = p_scn.tile([128, SL], bf, tag="scn", name="scn")
            nc.vector.tensor_mul(qn[:], raw_q[m][:], bq_sb[:, :SL])
            nc.gpsimd.dma_start(cc2_in[m * 128:(m + 1) * 128, :], qn[:])

        nc.gpsimd.collective_compute(
            "AllGather", mybir.AluOpType.bypass,
            ins=[cc2_in[0:QLR // 2, :]], outs=[cc2a_out[:]],
            replica_groups=[list(range(NC_N))],
        )
        nc.gpsimd.collective_compute(
            "AllGather", mybir.AluOpType.bypass,
            ins=[cc2_in[QLR // 2:QLR, :]], outs=[cc2b_out[:]],
            replica_groups=[list(range(NC_N))],
        )

        # --- ckv + k_pe pass ---
        ckv_ps = [pp_mm.tile([128, SL], f32, tag="mm", name="mm")
                  for _ in range(KC)]
        pe_ps = pp_mm.tile([128, SL], f32, tag="mm", name="mm")
        rot_ps = pp_mm.tile([128, SL], f32, tag="mm", name="mm")
        for k in range(KH):
            hst = p_hs.tile([128, SL], bf, tag="hs", name="hs")
            nc.sync.dma_start(hst[:], hsT_d[k * 128:(k + 1) * 128, :])
            wkv = p_w.tile([128, KVLR], bf, tag="wkv", name="wkv", bufs=3)
            nc.scalar.dma_start(wkv[:], wkvc_d[k * 128:(k + 1) * 128, :])
            wpr = p_w.tile([128, 256], bf, tag="wpr", name="wpr", bufs=3)
            nc.scalar.dma_start(wpr[:], wkperot_d[k * 128:(k + 1) * 128, :])
            for c in range(KC):
                nc.tensor.matmul(ckv_ps[c][:], wkv[:, c * 128:(c + 1) * 128],
                                 hst[:], start=(k == 0), stop=(k == KH - 1))
            nc.tensor.matmul(pe_ps[:], wpr[:, 0:128], hst[:],
                             start=(k == 0), stop=(k == KH - 1))
            nc.tensor.matmul(rot_ps[:], wpr[:, 128:256], hst[:],
                             start=(k == 0), stop=(k == KH - 1))
        ssq_kv = pp_sm.tile([1, SL], f32, tag="sm", name="sm")
        raw_kv = []
        for c in range(KC):
            r = p_raw.tile([128, SL], bf, tag="raw", name="raw")
            nc.scalar.activation(r[:], ckv_ps[c][:], AF.Copy)
            raw_kv.append(r)
            sq = p_sq.tile([128, SL], bf, tag="sq", name="sq")
            nc.scalar.activation(sq[:], ckv_ps[c][:], AF.Square)
            nc.tensor.matmul(ssq_kv[:], ones_col[:], sq[:],
                             start=(c == 0), stop=(c == KC - 1))
        t_kv = p_sml.tile([1, SL], f32, tag="sml", name="sml")
        nc.scalar.activation(t_kv[:], ssq_kv[:], AF.Sqrt,
                             bias=eps_t[:], scale=1.0 / KVLR)
        s_kv = p_sml.tile([1, SL], f32, tag="sml", name="sml")
        nc.vector.reciprocal(s_kv[:], t_kv[:])
        bkv_sb = p_bc.tile([128, 512], f32, tag="bc", name="bc")
        nc.gpsimd.partition_broadcast(bkv_sb[:, :SL], s_kv[:])
        for c in range(KC):
            cn = p_scn.tile([128, SL], bf, tag="scn", name="scn")
            nc.vector.tensor_mul(cn[:], raw_kv[c][:], bkv_sb[:, :SL])
            nc.gpsimd.dma_start(cc1_in[c * 128:(c + 1) * 128, :], cn[:])
        # rope k_pe
        cosl_t = p_csl.tile([128, SL], bf, tag="csl", name="csl")
        sinl_t = p_csl.tile([128, SL], bf, tag="csl", name="csl")
        nc.sync.dma_start(cosl_t[:], cosl_d[:, :])
        nc.sync.dma_start(sinl_t[:], sinl_d[:, :])
        t1 = p_f32.tile([128, SL], f32, tag="f32", name="f32")
        t2 = p_f32.tile([128, SL], f32, tag="f32", name="f32")
        nc.vector.tensor_mul(t1[:], pe_ps[:], cosl_t[:])
        nc.vector.tensor_mul(t2[:], rot_ps[:], sinl_t[:])
        kpe_n = p_scn.tile([128, SL], bf, tag="scn", name="scn")
        nc.vector.tensor_add(kpe_n[:], t1[:], t2[:])
        nc.gpsimd.dma_start(cc1_in[KVLR:KVLR + 128, :], kpe_n[:])

        nc.gpsimd.collective_compute(
            "AllGather", mybir.AluOpType.bypass,
            ins=[cc1_in[:]], outs=[cc1_out[:]],
            replica_groups=[list(range(NC_N))],
        )

        # resident back-end weights: prefetch on sync while AGs fly
        kc_t = {}
        for i in range(HPC):
            for c in range(KC):
                t = p_kc.tile([128, DN], bf, tag="kc", name="kc")
                nc.sync.dma_start(
                    t[:], kct_d[i * KVLR + c * 128:i * KVLR + (c + 1) * 128, :])
                kc_t[(i, c)] = t
        vc_t = {}
        for c in range(KC):
            t = p_vc.tile([128, HPC * DV], bf, tag="vc", name="vc")
            nc.sync.dma_start(t[:], vcp_d[c * 128:(c + 1) * 128, :])
            vc_t[c] = t
        wqb_t = []
        for k in range(KQ):
            t = p_wqb.tile([128, 1024], bf, tag="wqb", name="wqb")
            nc.sync.dma_start(t[:], wqb_d[k * 128:(k + 1) * 128, :])
            wqb_t.append(t)
        cosf_t = p_cs.tile([128, S], bf, tag="cs", name="cs")
        sinf_t = p_cs.tile([128, S], bf, tag="cs", name="cs")
        nc.sync.dma_start(cosf_t[:], cosf_d[:, :])
        nc.sync.dma_start(sinf_t[:], sinf_d[:, :])
        mask_t = []
        for m in range(4):
            t = p_msk.tile([128, 512], bf, tag="msk", name="msk")
            nc.sync.dma_start(t[:], masks_d[m * 128:(m + 1) * 128, :])
            mask_t.append(t)

        # ---------------- BACK: head-sharded attention ---------------------
        RPC = 512 // SL     # AG rank-blocks per 512-wide seq chunk
        ckvg = {}
        for c in range(KC):
            for sc in range(SC):
                t = p_ckvg.tile([128, 512], bf, tag="ckvg", name="ckvg")
                for half in range(RPC):
                    r = RPC * sc + half
                    nc.scalar.dma_start(
                        t[:, half * SL:(half + 1) * SL],
                        cc1_out[r * (KVLR + 128) + c * 128:
                                r * (KVLR + 128) + (c + 1) * 128, :])
                ckvg[(c, sc)] = t
        kpeg = {}
        for sc in range(SC):
            t = p_kpeg.tile([128, 512], bf, tag="kpeg", name="kpeg")
            for half in range(RPC):
                r = RPC * sc + half
                nc.scalar.dma_start(
                    t[:, half * SL:(half + 1) * SL],
                    cc1_out[r * (KVLR + 128) + KVLR:
                            r * (KVLR + 128) + KVLR + 128, :])
            kpeg[sc] = t

        # K^T per head: [DN, S]
        K_t = []
        for i in range(HPC):
            kt = p_K.tile([128, S], bf, tag="K", name="K")
            K_t.append(kt)
            for sc in range(SC):
                ps = pp_mm.tile([128, 512], f32, tag="mm", name="mm")
                for c in range(KC):
                    nc.tensor.matmul(ps[:], kc_t[(i, c)][:], ckvg[(c, sc)][:],
                                     start=(c == 0), stop=(c == KC - 1))
                nc.scalar.activation(kt[:, sc * 512:(sc + 1) * 512], ps[:],
                                     AF.Copy)

        # V natural: per seq-block [128, 4*DV]
        V_t = []
        for sb in range(SB):
            ps = pp_mm.tile([128, 512], f32, tag="mm", name="mm")
            for c in range(KC):
                nc.tensor.matmul(
                    ps[:],
                    ckvg[(c, sb // 4)][:, (sb % 4) * 128:(sb % 4 + 1) * 128],
                    vc_t[c][:], start=(c == 0), stop=(c == KC - 1))
            vt = p_V.tile([128, HPC * DV], bf, tag="V", name="V")
            nc.scalar.activation(vt[:], ps[:], AF.Copy)
            V_t.append(vt)

        oT = {}   # (head, sc) -> [DV, 512] bf16 (normalized o transposed)
        for sc in range(SC):
            # gathered q_lora^T tiles for this seq chunk
            qlg = []
            for k in range(KQ):
                t = p_qlg.tile([128, 512], bf, tag="qlg", name="qlg")
                buf = cc2a_out if k < KQ // 2 else cc2b_out
                kk = k if k < KQ // 2 else k - KQ // 2
                for half in range(RPC):
                    r = RPC * sc + half
                    nc.sync.dma_start(
                        t[:, half * SL:(half + 1) * SL],
                        buf[r * (QLR // 2) + kk * 128:
                            r * (QLR // 2) + (kk + 1) * 128, :])
                qlg.append(t)
            # Q^T nope per head (transient)
            qn_t = []
            for i in range(HPC):
                ps = pp_mm.tile([128, 512], f32, tag="mm", name="mm")
                for k in range(KQ):
                    nc.tensor.matmul(ps[:], wqb_t[k][:, i * 128:(i + 1) * 128],
                                     qlg[k][:], start=(k == 0),
                                     stop=(k == KQ - 1))
                qt = p_Qn.tile([128, 512], bf, tag="Qn", name="Qn")
                nc.scalar.activation(qt[:], ps[:], AF.Copy)
                qn_t.append(qt)
            # Q^T pe packs + rope
            roped = []
            for pkt in range(2):
                ps_pe = pp_mm.tile([128, 512], f32, tag="mm", name="mm")
                ps_ro = pp_mm.tile([128, 512], f32, tag="mm", name="mm")
                for k in range(KQ):
                    nc.tensor.matmul(
                        ps_pe[:], wqb_t[k][:, 512 + pkt * 128:512 + (pkt + 1) * 128],
                        qlg[k][:], start=(k == 0), stop=(k == KQ - 1))
                    nc.tensor.matmul(
                        ps_ro[:], wqb_t[k][:, 768 + pkt * 128:768 + (pkt + 1) * 128],
                        qlg[k][:], start=(k == 0), stop=(k == KQ - 1))
                u1 = p_f32.tile([128, 512], f32, tag="rope32", name="rope32")
                u2 = p_f32.tile([128, 512], f32, tag="rope32", name="rope32")
                nc.vector.tensor_mul(u1[:], ps_pe[:],
                                     cosf_t[:, sc * 512:(sc + 1) * 512])
                nc.vector.tensor_mul(u2[:], ps_ro[:],
                                     sinf_t[:, sc * 512:(sc + 1) * 512])
                rp = p_rope.tile([128, 512], bf, tag="rope", name="rope")
                nc.vector.tensor_add(rp[:], u1[:], u2[:])
                roped.append(rp)

            # attention for each head on this seq chunk
            for i in range(HPC):
                pkt, hp = i // 2, i % 2
                o_ps = pp_mm.tile([128, 512], f32, tag="mm", name="mm")
                d_ps = pp_sm.tile([1, 512], f32, tag="sm", name="sm")
                nj = 4 * sc + 4
                for j in range(nj):
                    s_ps = pp_mm.tile([128, 512], f32, tag="mm", name="mm")
                    nc.tensor.matmul(s_ps[:],
                                     K_t[i][:, j * 128:(j + 1) * 128],
                                     qn_t[i][:], start=True, stop=False)
                    nc.tensor.matmul(
                        s_ps[:],
                        kpeg[j // 4][hp * 64:(hp + 1) * 64,
                                     (j % 4) * 128:(j % 4 + 1) * 128],
                        roped[pkt][hp * 64:(hp + 1) * 64, :],
                        start=False, stop=True)
                    pt = p_P.tile([128, 512], bf, tag="P", name="P")
                    if j >= 4 * sc:
                        pr = p_P.tile([128, 512], bf, tag="Praw", name="Praw", bufs=2)
                        nc.scalar.activation(pr[:], s_ps[:], AF.Exp)
                        nc.vector.tensor_mul(pt[:], pr[:],
                                             mask_t[j - 4 * sc][:])
                    else:
                        nc.scalar.activation(pt[:], s_ps[:], AF.Exp)
                    nc.tensor.matmul(d_ps[:], ones_col[:], pt[:],
                                     start=(j == 0), stop=(j == nj - 1))
                    nc.tensor.matmul(o_ps[:],
                                     V_t[j][:, i * DV:(i + 1) * DV], pt[:],
                                     start=(j == 0), stop=(j == nj - 1))
                dinv = p_sml.tile([1, 512], f32, tag="sml", name="sml")
                nc.vector.reciprocal(dinv[:], d_ps[:])
                bc_sb = p_bc.tile([128, 512], f32, tag="bc", name="bc")
                nc.gpsimd.partition_broadcast(bc_sb[:], dinv[:])
                ot = p_oT.tile([128, 512], bf, tag="oT", name="oT")
                nc.vector.tensor_mul(ot[:], o_ps[:], bc_sb[:])
                oT[(i, sc)] = ot

            # w_o partial for this seq chunk (overlaps later chunks' attention)
            for n in range(HID // 512):
                wo_t = []
                for i in range(HPC):
                    t = p_wo.tile([128, 512], bf, tag="wo", name="wo")
                    nc.sync.dma_start(
                        t[:], wo_d[i * DV:(i + 1) * DV, n * 512:(n + 1) * 512])
                    wo_t.append(t)
                for sbl in range(4):
                    sb = sc * 4 + sbl
                    ps = pp_mm.tile([128, 512], f32, tag="mm", name="mm")
                    for i in range(HPC):
                        nc.tensor.matmul(
                            ps[:], oT[(i, sc)][:, sbl * 128:(sbl + 1) * 128],
                            wo_t[i][:], start=(i == 0), stop=(i == HPC - 1))
                    ot2 = p_out.tile([128, 512], f32, tag="outst", name="outst")
                    nc.vector.tensor_copy(ot2[:], ps[:])
                    nc.scalar.dma_start(
                        out_d[sb * 128:(sb + 1) * 128, n * 512:(n + 1) * 512],
                        ot2[:])

    nc.compile()
    return nc


def _prep_inputs(inputs):
    """Host-side sharding + weight folding. Returns in_maps (list of 8 dicts)."""
    BF = _bf16()

    hs = np.asarray(inputs['hidden_states'], np.float32)
    pos = np.asarray(inputs['positions'])
    w_qa = np.asarray(inputs['w_qa'], np.float32)
    q_a_ln_w = np.asarray(inputs['q_a_ln_w'], np.float32)
    w_qb = np.asarray(inputs['w_qb'], np.float32)
    w_kva = np.asarray(inputs['w_kva'], np.float32)
    kv_a_ln_w = np.asarray(inputs['kv_a_ln_w'], np.float32)
    kc = np.asarray(inputs['kc'], np.float32)
    vc = np.asarray(inputs['vc'], np.float32)
    w_o = np.asarray(inputs['w_o'], np.float32)

    perm = np.concatenate([np.arange(0, DR, 2), np.arange(1, DR, 2)])
    inv_freq = 1.0 / (ROPE_BASE ** (np.arange(0, DR, 2, dtype=np.float64) / DR))
    freqs = pos.astype(np.float64)[None, :] * inv_freq[:, None]     # [32, S]
    cosT = np.cos(freqs).astype(np.float32)
    sinT = np.sin(freqs).astype(np.float32)
    cos128 = np.tile(cosT, (4, 1)).astype(BF)                        # [128, S]
    sin128 = np.tile(sinT, (4, 1)).astype(BF)

    scale = DQ ** -0.5
    w_qb_eff = ((w_qb * q_a_ln_w[:, None]) * scale).reshape(QLR, H, DQ)

    w_pe = w_kva[:, KVLR:][:, perm]
    w_pe_rot = np.concatenate([-w_pe[:, 32:], w_pe[:, :32]], 1)
    wkperot = np.concatenate([w_pe, w_pe, w_pe_rot, w_pe_rot],
                             1).astype(BF)                        # [HID, 256]

    kc_f = kc * kv_a_ln_w[None, None, :]
    vc_f = vc * kv_a_ln_w[None, :, None]

    masks = np.ones((4, 128, 512), np.float32)
    for p in range(4):
        for b in range(4):
            blk = masks[p][:, b * 128:(b + 1) * 128]
            if b < p:
                blk[:] = 0.0
            elif b == p:
                kr = np.arange(128)[:, None]
                qc = np.arange(128)[None, :]
                blk[kr > qc] = 0.0
    masks_b = masks.reshape(4 * 128, 512).astype(BF)

    wqa_b = w_qa.astype(BF)
    wkvc_b = w_kva[:, :KVLR].astype(BF)

    in_maps = []
    for core in range(NC_N):
        rows = slice(core * SL, (core + 1) * SL)
        h0 = core * HPC

        wqb_all = np.empty((QLR, 1024), np.float32)
        for i in range(HPC):
            wqb_all[:, i * 128:(i + 1) * 128] = w_qb_eff[:, h0 + i, :DN]
        for pkt in range(2):
            a, b = h0 + 2 * pkt, h0 + 2 * pkt + 1
            pe_a = w_qb_eff[:, a, DN:][:, perm]
            pe_b = w_qb_eff[:, b, DN:][:, perm]
            wqb_all[:, 512 + pkt * 128:512 + pkt * 128 + 64] = pe_a
            wqb_all[:, 512 + pkt * 128 + 64:512 + (pkt + 1) * 128] = pe_b
            rot_a = np.concatenate([-pe_a[:, 32:], pe_a[:, :32]], 1)
            rot_b = np.concatenate([-pe_b[:, 32:], pe_b[:, :32]], 1)
            wqb_all[:, 768 + pkt * 128:768 + pkt * 128 + 64] = rot_a
            wqb_all[:, 768 + pkt * 128 + 64:768 + (pkt + 1) * 128] = rot_b

        kct = np.concatenate([kc_f[h0 + i].T for i in range(HPC)], 0)
        vcp = np.concatenate([vc_f[h0 + i] for i in range(HPC)], 1)
        wo_sh = w_o[h0 * DV:(h0 + HPC) * DV, :]

        in_maps.append({
            "hsT": np.ascontiguousarray(hs[rows].T).astype(BF),
            "wqa": wqa_b,
            "wkvc": wkvc_b,
            "wkperot": wkperot,
            "cosl": np.ascontiguousarray(cos128[:, rows]),
            "sinl": np.ascontiguousarray(sin128[:, rows]),
            "cosf": cos128,
            "sinf": sin128,
            "wqb": wqb_all.astype(BF),
            "kct": kct.astype(BF),
            "vcp": vcp.astype(BF),
            "wo": wo_sh.astype(BF),
            "masks": masks_b,
        })
    return in_maps


def _get_program():
    if "nc" not in _CACHE:
        _CACHE["nc"] = _build_program()
    return _CACHE["nc"]


def run(inputs, trace=False, trace_kwargs=None):
    """Build (cached), run on 8 cores, return (output, BassKernelResults)."""
    from concourse.bass_utils import run_bass_kernel_spmd

    nc = _get_program()
    in_maps = _prep_inputs(inputs)
    res = run_bass_kernel_spmd(nc, in_maps, list(range(NC_N)),
                               trace=trace, **(trace_kwargs or {}))
    out = np.zeros((S, HID), np.float32)
    for r in res.results:
        out += r["out_partial"]
    return out, res


def kernel(**inputs) -> np.ndarray:
    out, _ = run(inputs, trace=False)
    return out



# revision 16
# speedup vs baseline: 1.2755x; 1.2755x over previous
"""Trainium2 Bass kernel for a DeepseekV2 decoder-layer attention block
(MLA prefill, fp32 reference) distributed across 8 NeuronCores.

Strategy (single NEFF, SPMD on 8 cores):
  - Sequence-shard the shared projections: each core computes ckv / k_pe
    (RMS-normed / roped) then q_lora for its 256 rows of the sequence, in
    transposed layout; two on-device AllGathers replicate them. ckv goes
    first so its AllGather flies under the q_lora GEMM; the q_lora
    AllGather flies under the K/V expansion.
  - Head-shard the rest (4 heads per core): q_b projection + RoPE, kc/vc
    expansion, causal attention (scores computed transposed so the attn@v
    matmul needs no transposes), and a row-shard of w_o.
  - Each core emits a partial [S, HID] bf16 output; the host sums the 8
    partials (the output all-reduce) to produce the full result.

Perf notes vs the v1 baseline (980us):
  - The attention inner loop is software-pipelined with a lookahead of 2
    j-blocks so the tensor engine never stalls on the scalar engine's exp.
  - The softmax denominator is accumulated on the vector engine (bf16
    running sum of P tiles) with a single ones-matmul per (head, chunk),
    instead of a 512-row ones-matmul per j-block.
  - The RoPE rotation halves are produced with partition-offset vector ops
    against sign-folded sin tables instead of duplicate sign-flipped GEMM
    chains (removes ~60k tensor-engine rows).
  - w_o is SBUF-resident (loaded once), output staging is bf16 via the
    scalar engine, and the final partial is written as bf16.
"""

import numpy as np

S, HID, H = 2048, 5120, 32
QLR, KVLR = 1536, 512
DN, DR, DV = 128, 64, 128
DQ = DN + DR
NC_N = 8
HPC = H // NC_N          # heads per core
SL = S // NC_N           # sequence rows per core (front end)
ROPE_BASE, EPS = 10000.0, 1e-6

_CACHE = {}


def _bf16():
    import ml_dtypes
    return np.dtype(ml_dtypes.bfloat16)


def _build_program():
    import concourse.bass as bass
    import concourse.tile as tile
    from concourse import bacc, mybir
    from contextlib import ExitStack

    f32 = mybir.dt.float32
    bf = mybir.dt.bfloat16
    AF = mybir.ActivationFunctionType

    nc = bacc.Bacc("TRN2", target_bir_lowering=False, debug=False,
                   num_devices=NC_N)

    def din(name, shape, dt=bf):
        return nc.dram_tensor(name, list(shape), dt, kind="ExternalInput").ap()

    hsT_d = din("hsT", (HID, SL))
    wqa_d = din("wqa", (HID, QLR))
    wkvc_d = din("wkvc", (HID, KVLR))
    wkpe_d = din("wkpe", (HID, 128))         # [w_pe, w_pe] (perm'd)
    cosl_d = din("cosl", (128, SL))
    sinlsg_d = din("sinlsg", (128, SL))      # sign-folded sin
    cosf_d = din("cosf", (128, S))
    sinfsg_d = din("sinfsg", (128, S))
    wqb_d = din("wqb", (QLR, 768))           # nope(4x128) | pe(2x128, perm'd)
    kct_d = din("kct", (HPC * KVLR, DN))     # per head: kc'^T [KVLR, DN]
    vcp_d = din("vcp", (KVLR, HPC * DV))
    wo_d = din("wo", (HPC * DV, HID))
    masks_d = din("masks", (4 * 128, 512))   # 0/1 causal masks (scores^T)
    out_d = nc.dram_tensor("out_partial", [S, HID], bf,
                           kind="ExternalOutput").ap()

    cc1_in = nc.dram_tensor("cc1_in", [KVLR + 128, SL], bf).ap()
    cc1_out = nc.dram_tensor("cc1_out", [NC_N * (KVLR + 128), SL], bf,
                             addr_space="Shared").ap()
    cc2_in = nc.dram_tensor("cc2_in", [QLR, SL], bf).ap()
    cc2a_out = nc.dram_tensor("cc2a_out", [NC_N * (QLR // 2), SL], bf,
                              addr_space="Shared").ap()
    cc2b_out = nc.dram_tensor("cc2b_out", [NC_N * (QLR // 2), SL], bf,
                              addr_space="Shared").ap()

    KH = HID // 128       # 40 k-chunks of the model dim
    KQ = QLR // 128       # 12 chunks of the q-lora dim
    KC = KVLR // 128      # 4 chunks of the kv-lora dim
    SC = S // 512         # 4 sequence chunks of 512
    SB = S // 128         # 16 sequence blocks of 128

    with tile.TileContext(nc) as tc, ExitStack() as ctx:
        def pool(name, bufs):
            return ctx.enter_context(tc.tile_pool(name=name, bufs=bufs))

        p_hs = pool("hs", 8)
        p_w = pool("wstr", 3)
        p_raw = pool("raw", 16)
        p_sq = pool("sqt", 2)
        p_scn = pool("scn", 3)
        p_sml = pool("sml", 4)
        p_one = pool("ones", 2)
        p_cs = pool("cs", 2)
        p_csl = pool("csl", 2)
        p_bc = pool("bc", 2)
        p_kc = pool("kc", 16)
        p_vc = pool("vc", 4)
        p_wqb = pool("wqb", 12)
        p_wo = pool("wo", 40)
        p_msk = pool("msk", 4)
        p_kpeg = pool("kpeg", 4)
        p_K = pool("Kt", 4)
        p_V = pool("Vt", 16)
        p_qn = pool("Qn", 6)
        p_rope = pool("rope", 2)
        p_f32 = pool("fr32", 2)
        p_ckvg = pool("ckvg", 16)
        p_wk = pool("wk", 16)       # shared ring: qlg -> P tiles
        p_pacc = pool("pacc", 2)
        p_oT = pool("oT", 4)
        p_out = pool("outst", 3)

        pp_o = ctx.enter_context(
            tc.tile_pool(name="ppo", bufs=4, space="PSUM"))
        pp_s = ctx.enter_context(
            tc.tile_pool(name="pps", bufs=3, space="PSUM"))
        pp_sm = ctx.enter_context(
            tc.tile_pool(name="psm", bufs=1, space="PSUM"))

        ones_col = p_one.tile([128, 1], bf)       # lhsT for column sums
        nc.vector.memset(ones_col[:], 1.0)
        eps_t = p_one.tile([1, 1], f32, tag="eps", name="eps")
        nc.vector.memset(eps_t[:], EPS)

        def rope_apply(dst, src_ps, cos_t, sin_t, c0, width, u1, u2):
            """dst(bf16) = src*cos + rot(src)*sin_signed, via partition-offset
            muls. src_ps is a [128, width] f32 PSUM pack of 64-dim halves."""
            nc.vector.tensor_mul(u1[:, :width], src_ps[:],
                                 cos_t[:, c0:c0 + width])
            for blk in (0, 64):
                nc.vector.tensor_mul(
                    u2[blk:blk + 32, :width],
                    src_ps[blk + 32:blk + 64, :],
                    sin_t[blk:blk + 32, c0:c0 + width])
                nc.vector.tensor_mul(
                    u2[blk + 32:blk + 64, :width],
                    src_ps[blk:blk + 32, :],
                    sin_t[blk + 32:blk + 64, c0:c0 + width])
            nc.vector.tensor_add(dst, u1[:, :width], u2[:, :width])

        # ---------------- FRONT 1: ckv + k_pe pass (then AG1) --------------
        ckv_ps = [pp_o.tile([128, SL], f32, tag="po", name="po")
                  for _ in range(KC)]
        pe_ps = pp_s.tile([128, SL], f32, tag="ps", name="ps")
        for k in range(KH):
            hst = p_hs.tile([128, SL], bf, tag="hs", name="hs")
            nc.sync.dma_start(hst[:], hsT_d[k * 128:(k + 1) * 128, :])
            wkv = p_w.tile([128, KVLR], bf, tag="wkv", name="wkv", bufs=3)
            nc.scalar.dma_start(wkv[:], wkvc_d[k * 128:(k + 1) * 128, :])
            wpe = p_w.tile([128, 128], bf, tag="wpe", name="wpe", bufs=3)
            nc.scalar.dma_start(wpe[:], wkpe_d[k * 128:(k + 1) * 128, :])
            for c in range(KC):
                nc.tensor.matmul(ckv_ps[c][:], wkv[:, c * 128:(c + 1) * 128],
                                 hst[:], start=(k == 0), stop=(k == KH - 1))
            nc.tensor.matmul(pe_ps[:], wpe[:], hst[:],
                             start=(k == 0), stop=(k == KH - 1))
        ssq_kv = pp_sm.tile([1, SL], f32, tag="sm", name="sm")
        raw_kv = []
        for c in range(KC):
            r = p_raw.tile([128, SL], bf, tag="raw", name="raw")
            nc.scalar.activation(r[:], ckv_ps[c][:], AF.Copy)
            raw_kv.append(r)
            sq = p_sq.tile([128, SL], bf, tag="sq", name="sq")
            nc.scalar.activation(sq[:], ckv_ps[c][:], AF.Square)
            nc.tensor.matmul(ssq_kv[:], ones_col[:], sq[:],
                             start=(c == 0), stop=(c == KC - 1))
        t_kv = p_sml.tile([1, SL], f32, tag="sml", name="sml")
        nc.scalar.activation(t_kv[:], ssq_kv[:], AF.Sqrt,
                             bias=eps_t[:], scale=1.0 / KVLR)
        s_kv = p_sml.tile([1, SL], f32, tag="sml", name="sml")
        nc.vector.reciprocal(s_kv[:], t_kv[:])
        bkv_sb = p_bc.tile([128, 512], f32, tag="bc", name="bc")
        nc.gpsimd.partition_broadcast(bkv_sb[:, :SL], s_kv[:])
        for c in range(KC):
            cn = p_scn.tile([128, SL], bf, tag="scn", name="scn")
            nc.vector.tensor_mul(cn[:], raw_kv[c][:], bkv_sb[:, :SL])
            nc.gpsimd.dma_start(cc1_in[c * 128:(c + 1) * 128, :], cn[:])
        # rope k_pe (partition-offset rotation, sign folded into sin table)
        cosl_t = p_csl.tile([128, SL], bf, tag="csl", name="csl")
        sinl_t = p_csl.tile([128, SL], bf, tag="csl", name="csl")
        nc.sync.dma_start(cosl_t[:], cosl_d[:, :])
        nc.sync.dma_start(sinl_t[:], sinlsg_d[:, :])
        u1f = p_f32.tile([128, 512], f32, tag="f32", name="f32")
        u2f = p_f32.tile([128, 512], f32, tag="f32", name="f32")
        kpe_n = p_scn.tile([128, SL], bf, tag="scn", name="scn")
        rope_apply(kpe_n[:], pe_ps, cosl_t, sinl_t, 0, SL, u1f, u2f)
        nc.gpsimd.dma_start(cc1_in[KVLR:KVLR + 128, :], kpe_n[:])

        nc.gpsimd.collective_compute(
            "AllGather", mybir.AluOpType.bypass,
            ins=[cc1_in[:]], outs=[cc1_out[:]],
            replica_groups=[list(range(NC_N))],
        )

        # ---------------- FRONT 2: q_lora pass (then AG2) ------------------
        # resident back-end weights stream on the scalar ring meanwhile
        kc_t = {}
        for i in range(HPC):
            for c in range(KC):
                t = p_kc.tile([128, DN], bf, tag="kc", name="kc")
                nc.scalar.dma_start(
                    t[:], kct_d[i * KVLR + c * 128:i * KVLR + (c + 1) * 128, :])
                kc_t[(i, c)] = t
        vc_t = {}
        for c in range(KC):
            t = p_vc.tile([128, HPC * DV], bf, tag="vc", name="vc")
            nc.scalar.dma_start(t[:], vcp_d[c * 128:(c + 1) * 128, :])
            vc_t[c] = t
        wqb_t = []
        for k in range(KQ):
            t = p_wqb.tile([128, 768], bf, tag="wqb", name="wqb")
            nc.scalar.dma_start(t[:], wqb_d[k * 128:(k + 1) * 128, :])
            wqb_t.append(t)
        cosf_t = p_cs.tile([128, S], bf, tag="cs", name="cs")
        sinf_t = p_cs.tile([128, S], bf, tag="cs", name="cs")
        nc.scalar.dma_start(cosf_t[:], cosf_d[:, :])
        nc.scalar.dma_start(sinf_t[:], sinfsg_d[:, :])
        mask_t = []
        for m in range(4):
            t = p_msk.tile([128, 512], bf, tag="msk", name="msk")
            nc.scalar.dma_start(t[:], masks_d[m * 128:(m + 1) * 128, :])
            mask_t.append(t)

        ssq_q = pp_sm.tile([1, SL], f32, tag="sm", name="sm")
        raw_q = []
        for g in range(2):
            ql_ps = ([pp_o.tile([128, SL], f32, tag="po", name="po")
                      for _ in range(4)] +
                     [pp_s.tile([128, SL], f32, tag="ps", name="ps")
                      for _ in range(2)])
            for k in range(KH):
                hst = p_hs.tile([128, SL], bf, tag="hs", name="hs")
                nc.gpsimd.dma_start(hst[:], hsT_d[k * 128:(k + 1) * 128, :])
                w = p_w.tile([128, QLR // 2], bf, tag="wqa", name="wqa", bufs=3)
                nc.sync.dma_start(
                    w[:], wqa_d[k * 128:(k + 1) * 128,
                                g * (QLR // 2):(g + 1) * (QLR // 2)])
                for mi in range(6):
                    nc.tensor.matmul(ql_ps[mi][:],
                                     w[:, mi * 128:(mi + 1) * 128], hst[:],
                                     start=(k == 0), stop=(k == KH - 1))
            for mi in range(6):
                m = g * 6 + mi
                r = p_raw.tile([128, SL], bf, tag="raw", name="raw")
                nc.scalar.activation(r[:], ql_ps[mi][:], AF.Copy)
                raw_q.append(r)
                sq = p_sq.tile([128, SL], bf, tag="sq", name="sq")
                nc.scalar.activation(sq[:], ql_ps[mi][:], AF.Square)
                nc.tensor.matmul(ssq_q[:], ones_col[:], sq[:],
                                 start=(m == 0), stop=(m == KQ - 1))
        t_q = p_sml.tile([1, SL], f32, tag="sml", name="sml")
        nc.scalar.activation(t_q[:], ssq_q[:], AF.Sqrt,
                             bias=eps_t[:], scale=1.0 / QLR)
        s_q = p_sml.tile([1, SL], f32, tag="sml", name="sml")
        nc.vector.reciprocal(s_q[:], t_q[:])
        bq_sb = p_bc.tile([128, 512], f32, tag="bc", name="bc")
        nc.gpsimd.partition_broadcast(bq_sb[:, :SL], s_q[:])
        for m in range(KQ):
            qn = p_scn.tile([128, SL], bf, tag="scn", name="scn")
            nc.vector.tensor_mul(qn[:], raw_q[m][:], bq_sb[:, :SL])
            nc.gpsimd.dma_start(cc2_in[m * 128:(m + 1) * 128, :], qn[:])

        nc.gpsimd.collective_compute(
            "AllGather", mybir.AluOpType.bypass,
            ins=[cc2_in[0:QLR // 2, :]], outs=[cc2a_out[:]],
            replica_groups=[list(range(NC_N))],
        )
        nc.gpsimd.collective_compute(
            "AllGather", mybir.AluOpType.bypass,
            ins=[cc2_in[QLR // 2:QLR, :]], outs=[cc2b_out[:]],
            replica_groups=[list(range(NC_N))],
        )

        # ---------------- BACK: K/V expansion (needs AG1 only) -------------
        RPC = 512 // SL     # AG rank-blocks per 512-wide seq chunk
        ckvg = {}
        for c in range(KC):
            for sc in range(SC):
                t = p_ckvg.tile([128, 512], bf, tag="ckvg", name="ckvg")
                for half in range(RPC):
                    r = RPC * sc + half
                    nc.scalar.dma_start(
                        t[:, half * SL:(half + 1) * SL],
                        cc1_out[r * (KVLR + 128) + c * 128:
                                r * (KVLR + 128) + (c + 1) * 128, :])
                ckvg[(c, sc)] = t
        kpeg = {}
        for sc in range(SC):
            t = p_kpeg.tile([128, 512], bf, tag="kpeg", name="kpeg")
            for half in range(RPC):
                r = RPC * sc + half
                nc.scalar.dma_start(
                    t[:, half * SL:(half + 1) * SL],
                    cc1_out[r * (KVLR + 128) + KVLR:
                            r * (KVLR + 128) + KVLR + 128, :])
            kpeg[sc] = t
        # w_o resident tiles: streamed after the gathers, needed only once
        # the first w_o stage runs
        wo_t = {}
        for i in range(HPC):
            for n in range(HID // 512):
                t = p_wo.tile([128, 512], bf, tag="wo", name="wo")
                nc.scalar.dma_start(
                    t[:], wo_d[i * DV:(i + 1) * DV, n * 512:(n + 1) * 512])
                wo_t[(i, n)] = t

        # K^T per head: [DN, S]
        K_t = []
        for i in range(HPC):
            kt = p_K.tile([128, S], bf, tag="K", name="K")
            K_t.append(kt)
            for sc in range(SC):
                ps = pp_s.tile([128, 512], f32, tag="ps", name="ps")
                for c in range(KC):
                    nc.tensor.matmul(ps[:], kc_t[(i, c)][:], ckvg[(c, sc)][:],
                                     start=(c == 0), stop=(c == KC - 1))
                nc.scalar.activation(kt[:, sc * 512:(sc + 1) * 512], ps[:],
                                     AF.Copy)

        # V natural: per seq-block [128, 4*DV]
        V_t = []
        for sb in range(SB):
            ps = pp_s.tile([128, 512], f32, tag="ps", name="ps")
            for c in range(KC):
                nc.tensor.matmul(
                    ps[:],
                    ckvg[(c, sb // 4)][:, (sb % 4) * 128:(sb % 4 + 1) * 128],
                    vc_t[c][:], start=(c == 0), stop=(c == KC - 1))
            vt = p_V.tile([128, HPC * DV], bf, tag="V", name="V")
            nc.scalar.activation(vt[:], ps[:], AF.Copy)
            V_t.append(vt)

        # ---------------- BACK: per-chunk Q proj + attention + w_o ---------
        def gather_qlg(sc):
            qlg = []
            for k in range(KQ):
                t = p_wk.tile([128, 512], bf, tag="wk", name="wk")
                buf = cc2a_out if k < KQ // 2 else cc2b_out
                kk = k if k < KQ // 2 else k - KQ // 2
                for half in range(RPC):
                    r = RPC * sc + half
                    eng = nc.sync if half == 0 else nc.gpsimd
                    eng.dma_start(
                        t[:, half * SL:(half + 1) * SL],
                        buf[r * (QLR // 2) + kk * 128:
                            r * (QLR // 2) + (kk + 1) * 128, :])
                qlg.append(t)
            return qlg

        qlg = gather_qlg(0)
        for sc in range(SC):
            # --- Q^T nope per head + pe packs (roped) ---
            qn_t = []
            for i in range(HPC):
                ps = pp_o.tile([128, 512], f32, tag="po", name="po")
                for k in range(KQ):
                    nc.tensor.matmul(ps[:], wqb_t[k][:, i * 128:(i + 1) * 128],
                                     qlg[k][:], start=(k == 0),
                                     stop=(k == KQ - 1))
                qt = p_qn.tile([128, 512], bf, tag="Qn", name="Qn")
                nc.scalar.activation(qt[:], ps[:], AF.Copy)
                qn_t.append(qt)
            roped = []
            for pkt in range(2):
                ps_pe = pp_s.tile([128, 512], f32, tag="ps", name="ps")
                for k in range(KQ):
                    nc.tensor.matmul(
                        ps_pe[:], wqb_t[k][:, 512 + pkt * 128:
                                           512 + (pkt + 1) * 128],
                        qlg[k][:], start=(k == 0), stop=(k == KQ - 1))
                u1 = p_f32.tile([128, 512], f32, tag="f32", name="f32")
                u2 = p_f32.tile([128, 512], f32, tag="f32", name="f32")
                rp = p_rope.tile([128, 512], bf, tag="rope", name="rope")
                rope_apply(rp[:], ps_pe, cosf_t, sinf_t, sc * 512, 512, u1, u2)
                roped.append(rp)

            # --- attention, software-pipelined over j (lookahead 2) ---
            o_ps_l, dinv_l = [], []
            for i in range(HPC):
                pkt, hp = i // 2, i % 2
                o_ps = pp_o.tile([128, 512], f32, tag="po", name="po")
                o_ps_l.append(o_ps)
                pacc = p_pacc.tile([128, 512], bf, tag="pacc", name="pacc")
                nj = 4 * sc + 4
                pts = []

                def consume(j):
                    nc.tensor.matmul(o_ps[:],
                                     V_t[j][:, i * DV:(i + 1) * DV],
                                     pts[j][:], start=(j == 0),
                                     stop=(j == nj - 1))
                    if j == 0:
                        nc.vector.tensor_copy(pacc[:], pts[j][:])
                    else:
                        nc.vector.tensor_add(pacc[:], pacc[:], pts[j][:])

                for j in range(nj):
                    s_ps = pp_s.tile([128, 512], f32, tag="ps", name="ps")
                    nc.tensor.matmul(s_ps[:],
                                     K_t[i][:, j * 128:(j + 1) * 128],
                                     qn_t[i][:], start=True, stop=False)
                    nc.tensor.matmul(
                        s_ps[:],
                        kpeg[j // 4][hp * 64:(hp + 1) * 64,
                                     (j % 4) * 128:(j % 4 + 1) * 128],
                        roped[pkt][hp * 64:(hp + 1) * 64, :],
                        start=False, stop=True)
                    if j >= 4 * sc:
                        pr = p_wk.tile([128, 512], bf, tag="wk", name="wk")
                        nc.scalar.activation(pr[:], s_ps[:], AF.Exp)
                        pt = p_wk.tile([128, 512], bf, tag="wk", name="wk")
                        nc.vector.tensor_mul(pt[:], pr[:],
                                             mask_t[j - 4 * sc][:])
                    else:
                        pt = p_wk.tile([128, 512], bf, tag="wk", name="wk")
                        nc.scalar.activation(pt[:], s_ps[:], AF.Exp)
                    pts.append(pt)
                    if j >= 2:
                        consume(j - 2)
                for j in range(max(0, nj - 2), nj):
                    consume(j)
                d_ps = pp_sm.tile([1, 512], f32, tag="sm", name="sm")
                nc.tensor.matmul(d_ps[:], ones_col[:], pacc[:],
                                 start=True, stop=True)
                dinv = p_sml.tile([1, 512], f32, tag="sml", name="sml")
                nc.vector.reciprocal(dinv[:], d_ps[:])
                dinv_l.append(dinv)

            oT = []
            for i in range(HPC):
                bc_sb = p_bc.tile([128, 512], f32, tag="bc", name="bc")
                nc.gpsimd.partition_broadcast(bc_sb[:], dinv_l[i][:])
                ot = p_oT.tile([128, 512], bf, tag="oT", name="oT")
                nc.vector.tensor_mul(ot[:], o_ps_l[i][:], bc_sb[:])
                oT.append(ot)

            # prefetch next chunk's gathered q_lora during w_o
            if sc + 1 < SC:
                qlg = gather_qlg(sc + 1)

            # --- w_o partial for this seq chunk ---
            for n in range(HID // 512):
                for sbl in range(4):
                    sb = sc * 4 + sbl
                    ps = pp_s.tile([128, 512], f32, tag="ps", name="ps")
                    for i in range(HPC):
                        nc.tensor.matmul(
                            ps[:], oT[i][:, sbl * 128:(sbl + 1) * 128],
                            wo_t[(i, n)][:], start=(i == 0),
                            stop=(i == HPC - 1))
                    ob = p_out.tile([128, 512], bf, tag="outst", name="outst")
                    nc.scalar.activation(ob[:], ps[:], AF.Copy)
                    nc.sync.dma_start(
                        out_d[sb * 128:(sb + 1) * 128, n * 512:(n + 1) * 512],
                        ob[:])

    nc.compile()
    return nc


def _prep_inputs(inputs):
    """Host-side sharding + weight folding. Returns in_maps (list of 8 dicts)."""
    BF = _bf16()

    hs = np.asarray(inputs['hidden_states'], np.float32)
    pos = np.asarray(inputs['positions'])
    w_qa = np.asarray(inputs['w_qa'], np.float32)
    q_a_ln_w = np.asarray(inputs['q_a_ln_w'], np.float32)
    w_qb = np.asarray(inputs['w_qb'], np.float32)
    w_kva = np.asarray(inputs['w_kva'], np.float32)
    kv_a_ln_w = np.asarray(inputs['kv_a_ln_w'], np.float32)
    kc = np.asarray(inputs['kc'], np.float32)
    vc = np.asarray(inputs['vc'], np.float32)
    w_o = np.asarray(inputs['w_o'], np.float32)

    perm = np.concatenate([np.arange(0, DR, 2), np.arange(1, DR, 2)])
    inv_freq = 1.0 / (ROPE_BASE ** (np.arange(0, DR, 2, dtype=np.float64) / DR))
    freqs = pos.astype(np.float64)[None, :] * inv_freq[:, None]     # [32, S]
    cosT = np.cos(freqs).astype(np.float32)
    sinT = np.sin(freqs).astype(np.float32)
    cos128 = np.tile(cosT, (4, 1)).astype(BF)                        # [128, S]
    sin128 = np.tile(sinT, (4, 1)).astype(np.float32)
    # sign-folded sin: rows r with (r % 64) < 32 are negated (they multiply
    # the rotated -x_hi half)
    sgn = np.where((np.arange(128) % 64) < 32, -1.0, 1.0)[:, None]
    sinsg128 = (sin128 * sgn).astype(BF)

    scale = DQ ** -0.5
    w_qb_eff = ((w_qb * q_a_ln_w[:, None]) * scale).reshape(QLR, H, DQ)

    w_pe = w_kva[:, KVLR:][:, perm]
    wkpe = np.concatenate([w_pe, w_pe], 1).astype(BF)             # [HID, 128]

    kc_f = kc * kv_a_ln_w[None, None, :]
    vc_f = vc * kv_a_ln_w[None, :, None]

    masks = np.ones((4, 128, 512), np.float32)
    for p in range(4):
        for b in range(4):
            blk = masks[p][:, b * 128:(b + 1) * 128]
            if b < p:
                blk[:] = 0.0
            elif b == p:
                kr = np.arange(128)[:, None]
                qc = np.arange(128)[None, :]
                blk[kr > qc] = 0.0
    masks_b = masks.reshape(4 * 128, 512).astype(BF)

    wqa_b = w_qa.astype(BF)
    wkvc_b = w_kva[:, :KVLR].astype(BF)

    in_maps = []
    for core in range(NC_N):
        rows = slice(core * SL, (core + 1) * SL)
        h0 = core * HPC

        wqb_all = np.empty((QLR, 768), np.float32)
        for i in range(HPC):
            wqb_all[:, i * 128:(i + 1) * 128] = w_qb_eff[:, h0 + i, :DN]
        for pkt in range(2):
            a, b = h0 + 2 * pkt, h0 + 2 * pkt + 1
            pe_a = w_qb_eff[:, a, DN:][:, perm]
            pe_b = w_qb_eff[:, b, DN:][:, perm]
            wqb_all[:, 512 + pkt * 128:512 + pkt * 128 + 64] = pe_a
            wqb_all[:, 512 + pkt * 128 + 64:512 + (pkt + 1) * 128] = pe_b

        kct = np.concatenate([kc_f[h0 + i].T for i in range(HPC)], 0)
        vcp = np.concatenate([vc_f[h0 + i] for i in range(HPC)], 1)
        wo_sh = w_o[h0 * DV:(h0 + HPC) * DV, :]

        in_maps.append({
            "hsT": np.ascontiguousarray(hs[rows].T).astype(BF),
            "wqa": wqa_b,
            "wkvc": wkvc_b,
            "wkpe": wkpe,
            "cosl": np.ascontiguousarray(cos128[:, rows]),
            "sinlsg": np.ascontiguousarray(sinsg128[:, rows]),
            "cosf": cos128,
            "sinfsg": sinsg128,
            "wqb": wqb_all.astype(BF),
            "kct": kct.astype(BF),
            "vcp": vcp.astype(BF),
            "wo": wo_sh.astype(BF),
            "masks": masks_b,
        })
    return in_maps


def _get_program():
    if "nc" not in _CACHE:
        _CACHE["nc"] = _build_program()
    return _CACHE["nc"]


def run(inputs, trace=False, trace_kwargs=None):
    """Build (cached), run on 8 cores, return (output, BassKernelResults)."""
    from concourse.bass_utils import run_bass_kernel_spmd

    nc = _get_program()
    in_maps = _prep_inputs(inputs)
    res = run_bass_kernel_spmd(nc, in_maps, list(range(NC_N)),
                               trace=trace, **(trace_kwargs or {}))
    out = np.zeros((S, HID), np.float32)
    for r in res.results:
        out += np.asarray(r["out_partial"], dtype=np.float32)
    return out, res


def kernel(**inputs) -> np.ndarray:
    out, _ = run(inputs, trace=False)
    return out


# revision 19
# speedup vs baseline: 1.3540x; 1.0615x over previous
"""Trainium2 Bass kernel for a DeepseekV2 decoder-layer attention block
(MLA prefill, fp32 reference) distributed across 8 NeuronCores.

Strategy (single NEFF, SPMD on 8 cores):
  - Sequence-shard the shared projections: each core computes ckv / k_pe
    (RMS-normed / roped) then q_lora for its 256 rows of the sequence, in
    transposed layout; two on-device AllGathers replicate them. ckv goes
    first so its AllGather flies under the q_lora GEMM; the q_lora
    AllGather flies under the K/V expansion.
  - Head-shard the rest (4 heads per core): q_b projection + RoPE, kc/vc
    expansion, causal attention (scores computed transposed so the attn@v
    matmul needs no transposes), and a row-shard of w_o.
  - Each core emits a partial [S, HID] bf16 output; the host sums the 8
    partials (the output all-reduce) to produce the full result.

Perf structure (v3):
  - All weight/activation streams are host-packed into [128, N] layouts so
    every DMA is one large transfer, spread round-robin across the
    sync/scalar/gpsimd/vector DGE rings (the per-DMA ~600ns issue cost made
    the v2 front end ring-bound).
  - Attention is software-pipelined (lookahead 2); the causal mask is
    applied as a third matmul into the score PSUM group (-1e30 * step),
    so the exp -> attn@v chain has no vector-engine hop.
  - Softmax denominators: bf16 P-tile running sum on the vector engine,
    one ones-matmul per (head, chunk), reciprocal_approx_fast.
  - RoPE rotation via partition-offset vector ops with sign-folded sin.
  - w_o resident; output staged bf16 through the scalar engine.
"""

import numpy as np

S, HID, H = 2048, 5120, 32
QLR, KVLR = 1536, 512
DN, DR, DV = 128, 64, 128
DQ = DN + DR
NC_N = 8
HPC = H // NC_N          # heads per core
SL = S // NC_N           # sequence rows per core (front end)
ROPE_BASE, EPS = 10000.0, 1e-6

_CACHE = {}


def _bf16():
    import ml_dtypes
    return np.dtype(ml_dtypes.bfloat16)


def _build_program():
    import concourse.bass as bass
    import concourse.tile as tile
    from concourse import bacc, mybir
    from contextlib import ExitStack

    f32 = mybir.dt.float32
    bf = mybir.dt.bfloat16
    AF = mybir.ActivationFunctionType

    nc = bacc.Bacc("TRN2", target_bir_lowering=False, debug=False,
                   num_devices=NC_N)

    def din(name, shape, dt=bf):
        return nc.dram_tensor(name, list(shape), dt, kind="ExternalInput").ap()

    KH = HID // 128       # 40 k-chunks of the model dim
    K2 = KH // 2          # 20 double-chunks
    KQ = QLR // 128       # 12 chunks of the q-lora dim
    KC = KVLR // 128      # 4 chunks of the kv-lora dim
    SC = S // 512         # 4 sequence chunks of 512
    SB = S // 128         # 16 sequence blocks of 128
    NW = HID // 512       # 10 w_o column chunks

    hsT2_d = din("hsT2", (128, K2 * 2 * SL))    # [p, k2*512+half*256+c]
    wqa_d = din("wqa", (HID, QLR))
    wkvpe_d = din("wkvpe", (HID, KVLR + 128))   # ckv cols | [w_pe, w_pe]
    cosl_d = din("cosl", (128, SL))
    sinlsg_d = din("sinlsg", (128, SL))         # sign-folded sin
    cosf_d = din("cosf", (128, S))
    sinfsg_d = din("sinfsg", (128, S))
    wqb2_d = din("wqb2", (128, KQ * 768))       # [p, k*768 + col]
    kct2_d = din("kct2", (128, HPC * KC * 128))  # [p, (i*4+c)*128 + d]
    vcp2_d = din("vcp2", (128, KC * HPC * DV))  # [p, c*512 + col]
    wo2_d = din("wo2", (128, HPC * NW * 512))   # [p, (i*NW+n)*512 + col]
    steps_d = din("steps", (128, 4 * 512))      # [r, p*512+q] = [p*128+r > q]
    negeye_d = din("negeye", (128, 128))        # -1e30 * I
    out_d = nc.dram_tensor("out_partial", [S, HID], bf,
                           kind="ExternalOutput").ap()

    cc1_in = nc.dram_tensor("cc1_in", [KVLR + 128, SL], bf).ap()
    cc1_out = nc.dram_tensor("cc1_out", [NC_N * (KVLR + 128), SL], bf,
                             addr_space="Shared").ap()
    cc2_in = nc.dram_tensor("cc2_in", [QLR, SL], bf).ap()
    cc2_out = nc.dram_tensor("cc2_out", [NC_N * QLR, SL], bf,
                             addr_space="Shared").ap()

    with tile.TileContext(nc) as tc, ExitStack() as ctx:
        def pool(name, bufs):
            return ctx.enter_context(tc.tile_pool(name=name, bufs=bufs))

        p_hs = pool("hs", 5)
        p_w = pool("wstr", 3)
        p_raw = pool("raw", 14)
        p_sq = pool("sqt", 2)
        p_scn = pool("scn", 3)
        p_sml = pool("sml", 4)
        p_one = pool("ones", 2)
        p_cs = pool("cs", 2)
        p_csl = pool("csl", 2)
        p_bc = pool("bc", 2)
        p_kc = pool("kc", 1)
        p_vc = pool("vc", 1)
        p_wqb = pool("wqb", 2)
        p_wo = pool("wo", 5)
        p_msk = pool("msk", 1)
        p_kpeg = pool("kpeg", 4)
        p_K = pool("Kt", 4)
        p_V = pool("Vt", 16)
        p_qn = pool("Qn", 5)
        p_rope = pool("rope", 2)
        p_f32 = pool("fr32", 2)
        p_ckvg = pool("ckvg", 16)
        p_wk = pool("wk", 16)       # shared ring: qlg -> P tiles
        p_pacc = pool("pacc", 2)
        p_oT = pool("oT", 4)
        p_out = pool("outst", 3)

        pp_o = ctx.enter_context(
            tc.tile_pool(name="ppo", bufs=4, space="PSUM"))
        pp_s = ctx.enter_context(
            tc.tile_pool(name="pps", bufs=3, space="PSUM"))
        pp_sm = ctx.enter_context(
            tc.tile_pool(name="psm", bufs=1, space="PSUM"))

        ones_col = p_one.tile([128, 1], bf)       # lhsT for column sums
        nc.vector.memset(ones_col[:], 1.0)
        eps_t = p_one.tile([1, 1], f32, tag="eps", name="eps")
        nc.vector.memset(eps_t[:], EPS)

        def rope_apply(dst, src_ps, cos_t, sin_t, c0, width, u1, u2):
            """dst(bf16) = src*cos + rot(src)*sin_signed, via partition-offset
            muls. src_ps is a [128, width] f32 PSUM pack of 64-dim halves."""
            nc.vector.tensor_mul(u1[:, :width], src_ps[:],
                                 cos_t[:, c0:c0 + width])
            for blk in (0, 64):
                nc.vector.tensor_mul(
                    u2[blk:blk + 32, :width],
                    src_ps[blk + 32:blk + 64, :],
                    sin_t[blk:blk + 32, c0:c0 + width])
                nc.vector.tensor_mul(
                    u2[blk + 32:blk + 64, :width],
                    src_ps[blk:blk + 32, :],
                    sin_t[blk + 32:blk + 64, c0:c0 + width])
            nc.vector.tensor_add(dst, u1[:, :width], u2[:, :width])

        # ---------------- FRONT 1: ckv + k_pe pass (then AG1) --------------
        ckv_ps = [pp_o.tile([128, SL], f32, tag="po", name="po")
                  for _ in range(KC)]
        pe_ps = pp_s.tile([128, SL], f32, tag="ps", name="ps")
        for k2 in range(K2):
            hs2 = p_hs.tile([128, 2 * SL], bf, tag="hs", name="hs")
            nc.sync.dma_start(hs2[:], hsT2_d[:, k2 * 512:(k2 + 1) * 512])
            wv = []
            for kk in range(2):
                k = 2 * k2 + kk
                w = p_w.tile([128, KVLR + 128], bf, tag="wkv", name="wkv",
                             bufs=3)
                eng = nc.scalar if kk == 0 else nc.gpsimd
                eng.dma_start(w[:], wkvpe_d[k * 128:(k + 1) * 128, :])
                wv.append(w)
            for kk in range(2):
                k = 2 * k2 + kk
                hh = hs2[:, kk * SL:(kk + 1) * SL]
                for c in range(KC):
                    nc.tensor.matmul(ckv_ps[c][:],
                                     wv[kk][:, c * 128:(c + 1) * 128],
                                     hh, start=(k == 0), stop=(k == KH - 1))
                nc.tensor.matmul(pe_ps[:], wv[kk][:, KVLR:KVLR + 128], hh,
                                 start=(k == 0), stop=(k == KH - 1))
        ssq_kv = pp_sm.tile([1, SL], f32, tag="sm", name="sm")
        raw_kv = []
        for c in range(KC):
            r = p_raw.tile([128, SL], bf, tag="raw", name="raw")
            nc.scalar.activation(r[:], ckv_ps[c][:], AF.Copy)
            raw_kv.append(r)
            sq = p_sq.tile([128, SL], bf, tag="sq", name="sq")
            nc.scalar.activation(sq[:], ckv_ps[c][:], AF.Square)
            nc.tensor.matmul(ssq_kv[:], ones_col[:], sq[:],
                             start=(c == 0), stop=(c == KC - 1))
        t_kv = p_sml.tile([1, SL], f32, tag="sml", name="sml")
        nc.scalar.activation(t_kv[:], ssq_kv[:], AF.Sqrt,
                             bias=eps_t[:], scale=1.0 / KVLR)
        s_kv = p_sml.tile([1, SL], f32, tag="sml", name="sml")
        nc.vector.reciprocal(s_kv[:], t_kv[:])
        bkv_sb = p_bc.tile([128, 512], f32, tag="bc", name="bc")
        nc.gpsimd.partition_broadcast(bkv_sb[:, :SL], s_kv[:])
        for c in range(KC):
            cn = p_scn.tile([128, SL], bf, tag="scn", name="scn")
            nc.vector.tensor_mul(cn[:], raw_kv[c][:], bkv_sb[:, :SL])
            nc.gpsimd.dma_start(cc1_in[c * 128:(c + 1) * 128, :], cn[:])
        # rope k_pe (partition-offset rotation, sign folded into sin table)
        cosl_t = p_csl.tile([128, SL], bf, tag="csl", name="csl")
        sinl_t = p_csl.tile([128, SL], bf, tag="csl", name="csl")
        nc.sync.dma_start(cosl_t[:], cosl_d[:, :])
        nc.sync.dma_start(sinl_t[:], sinlsg_d[:, :])
        u1f = p_f32.tile([128, 512], f32, tag="f32", name="f32")
        u2f = p_f32.tile([128, 512], f32, tag="f32", name="f32")
        kpe_n = p_scn.tile([128, SL], bf, tag="scn", name="scn")
        rope_apply(kpe_n[:], pe_ps, cosl_t, sinl_t, 0, SL, u1f, u2f)
        nc.gpsimd.dma_start(cc1_in[KVLR:KVLR + 128, :], kpe_n[:])

        nc.gpsimd.collective_compute(
            "AllGather", mybir.AluOpType.bypass,
            ins=[cc1_in[:]], outs=[cc1_out[:]],
            replica_groups=[list(range(NC_N))],
        )

        # ---------------- FRONT 2: q_lora pass (then AG2) ------------------
        # resident back-end weights stream on the scalar/gpsimd rings
        kct_t = p_kc.tile([128, HPC * KC * 128], bf, tag="kc", name="kc")
        nc.scalar.dma_start(kct_t[:], kct2_d[:, :])
        vcp_t = p_vc.tile([128, KC * HPC * DV], bf, tag="vc", name="vc")
        nc.gpsimd.dma_start(vcp_t[:], vcp2_d[:, :])
        wqb_t = []
        for h in range(2):
            t = p_wqb.tile([128, 6 * 768], bf, tag="wqb", name="wqb")
            eng = nc.scalar if h == 0 else nc.gpsimd
            eng.dma_start(t[:], wqb2_d[:, h * 6 * 768:(h + 1) * 6 * 768])
            wqb_t.append(t)
        cosf_t = p_cs.tile([128, S], bf, tag="cs", name="cs")
        sinf_t = p_cs.tile([128, S], bf, tag="cs", name="cs")
        nc.scalar.dma_start(cosf_t[:], cosf_d[:, :])
        nc.gpsimd.dma_start(sinf_t[:], sinfsg_d[:, :])
        steps_t = p_msk.tile([128, 4 * 512], bf, tag="msk", name="msk")
        nc.scalar.dma_start(steps_t[:], steps_d[:, :])
        negeye_t = p_msk.tile([128, 128], bf, tag="ne", name="ne")
        nc.gpsimd.dma_start(negeye_t[:], negeye_d[:, :])

        def wqb_ap(k, c0, c1):
            return wqb_t[k // 6][:, (k % 6) * 768 + c0:(k % 6) * 768 + c1]

        ssq_q = pp_sm.tile([1, SL], f32, tag="sm", name="sm")
        raw_q = []
        for g in range(2):
            ql_ps = ([pp_o.tile([128, SL], f32, tag="po", name="po")
                      for _ in range(4)] +
                     [pp_s.tile([128, SL], f32, tag="ps", name="ps")
                      for _ in range(2)])
            for k2 in range(K2):
                hs2 = p_hs.tile([128, 2 * SL], bf, tag="hs", name="hs")
                nc.gpsimd.dma_start(hs2[:], hsT2_d[:, k2 * 512:(k2 + 1) * 512])
                wq = []
                for kk in range(2):
                    k = 2 * k2 + kk
                    w = p_w.tile([128, QLR // 2], bf, tag="wqa", name="wqa",
                                 bufs=3)
                    eng = nc.sync if kk == 0 else nc.scalar
                    eng.dma_start(
                        w[:], wqa_d[k * 128:(k + 1) * 128,
                                    g * (QLR // 2):(g + 1) * (QLR // 2)])
                    wq.append(w)
                for kk in range(2):
                    k = 2 * k2 + kk
                    hh = hs2[:, kk * SL:(kk + 1) * SL]
                    for mi in range(6):
                        nc.tensor.matmul(ql_ps[mi][:],
                                         wq[kk][:, mi * 128:(mi + 1) * 128],
                                         hh, start=(k == 0),
                                         stop=(k == KH - 1))
            for mi in range(6):
                m = g * 6 + mi
                r = p_raw.tile([128, SL], bf, tag="raw", name="raw")
                nc.scalar.activation(r[:], ql_ps[mi][:], AF.Copy)
                raw_q.append(r)
                sq = p_sq.tile([128, SL], bf, tag="sq", name="sq")
                nc.scalar.activation(sq[:], ql_ps[mi][:], AF.Square)
                nc.tensor.matmul(ssq_q[:], ones_col[:], sq[:],
                                 start=(m == 0), stop=(m == KQ - 1))
        t_q = p_sml.tile([1, SL], f32, tag="sml", name="sml")
        nc.scalar.activation(t_q[:], ssq_q[:], AF.Sqrt,
                             bias=eps_t[:], scale=1.0 / QLR)
        s_q = p_sml.tile([1, SL], f32, tag="sml", name="sml")
        nc.vector.reciprocal(s_q[:], t_q[:])
        bq_sb = p_bc.tile([128, 512], f32, tag="bc", name="bc")
        nc.gpsimd.partition_broadcast(bq_sb[:, :SL], s_q[:])
        for m in range(KQ):
            qn = p_scn.tile([128, SL], bf, tag="scn", name="scn")
            nc.vector.tensor_mul(qn[:], raw_q[m][:], bq_sb[:, :SL])
            nc.gpsimd.dma_start(cc2_in[m * 128:(m + 1) * 128, :], qn[:])

        nc.gpsimd.collective_compute(
            "AllGather", mybir.AluOpType.bypass,
            ins=[cc2_in[:]], outs=[cc2_out[:]],
            replica_groups=[list(range(NC_N))],
        )

        # ---------------- BACK: K/V expansion (needs AG1 only) -------------
        RPC = 512 // SL     # AG rank-blocks per 512-wide seq chunk
        g_engs = [nc.scalar, nc.gpsimd, nc.sync]
        g_rr = [0]

        def rr_eng():
            e = g_engs[g_rr[0] % 3]
            g_rr[0] += 1
            return e

        ckvg = {}
        for c in range(KC):
            for sc in range(SC):
                t = p_ckvg.tile([128, 512], bf, tag="ckvg", name="ckvg")
                for half in range(RPC):
                    r = RPC * sc + half
                    rr_eng().dma_start(
                        t[:, half * SL:(half + 1) * SL],
                        cc1_out[r * (KVLR + 128) + c * 128:
                                r * (KVLR + 128) + (c + 1) * 128, :])
                ckvg[(c, sc)] = t
        kpeg = {}
        for sc in range(SC):
            t = p_kpeg.tile([128, 512], bf, tag="kpeg", name="kpeg")
            for half in range(RPC):
                r = RPC * sc + half
                rr_eng().dma_start(
                    t[:, half * SL:(half + 1) * SL],
                    cc1_out[r * (KVLR + 128) + KVLR:
                            r * (KVLR + 128) + KVLR + 128, :])
            kpeg[sc] = t
        # w_o resident tiles: streamed after the gathers
        wo_t = []
        for h in range(5):
            t = p_wo.tile([128, 8 * 512], bf, tag="wo", name="wo")
            eng = g_engs[h % 2]
            eng.dma_start(t[:], wo2_d[:, h * 8 * 512:(h + 1) * 8 * 512])
            wo_t.append(t)

        def wo_ap(i, n):
            b = i * NW + n
            return wo_t[b // 8][:, (b % 8) * 512:(b % 8 + 1) * 512]

        # K^T per head: [DN, S]
        K_t = []
        for i in range(HPC):
            kt = p_K.tile([128, S], bf, tag="K", name="K")
            K_t.append(kt)
            for sc in range(SC):
                ps = pp_s.tile([128, 512], f32, tag="ps", name="ps")
                for c in range(KC):
                    nc.tensor.matmul(
                        ps[:],
                        kct_t[:, (i * KC + c) * 128:(i * KC + c + 1) * 128],
                        ckvg[(c, sc)][:],
                        start=(c == 0), stop=(c == KC - 1))
                nc.scalar.activation(kt[:, sc * 512:(sc + 1) * 512], ps[:],
                                     AF.Copy)

        # V natural: per seq-block [128, 4*DV]
        V_t = []
        for sb in range(SB):
            ps = pp_s.tile([128, 512], f32, tag="ps", name="ps")
            for c in range(KC):
                nc.tensor.matmul(
                    ps[:],
                    ckvg[(c, sb // 4)][:, (sb % 4) * 128:(sb % 4 + 1) * 128],
                    vcp_t[:, c * 512:(c + 1) * 512],
                    start=(c == 0), stop=(c == KC - 1))
            vt = p_V.tile([128, HPC * DV], bf, tag="V", name="V")
            nc.scalar.activation(vt[:], ps[:], AF.Copy)
            V_t.append(vt)

        # ---------------- BACK: per-chunk Q proj + attention + w_o ---------
        qg_engs = [nc.gpsimd, nc.scalar]

        def gather_qlg(sc):
            qlg = []
            for k in range(KQ):
                t = p_wk.tile([128, 512], bf, tag="wk", name="wk")
                for half in range(RPC):
                    r = RPC * sc + half
                    qg_engs[(2 * k + half) % 2].dma_start(
                        t[:, half * SL:(half + 1) * SL],
                        cc2_out[r * QLR + k * 128:r * QLR + (k + 1) * 128, :])
                qlg.append(t)
            return qlg

        qlg = gather_qlg(0)
        for sc in range(SC):
            # --- Q^T nope per head + pe packs (roped) ---
            qn_t = []
            for i in range(HPC):
                ps = pp_o.tile([128, 512], f32, tag="po", name="po")
                for k in range(KQ):
                    nc.tensor.matmul(ps[:], wqb_ap(k, i * 128, (i + 1) * 128),
                                     qlg[k][:], start=(k == 0),
                                     stop=(k == KQ - 1))
                qt = p_qn.tile([128, 512], bf, tag="Qn", name="Qn")
                nc.scalar.activation(qt[:], ps[:], AF.Copy)
                qn_t.append(qt)
            roped = []
            for pkt in range(2):
                ps_pe = pp_s.tile([128, 512], f32, tag="ps", name="ps")
                for k in range(KQ):
                    nc.tensor.matmul(
                        ps_pe[:],
                        wqb_ap(k, 512 + pkt * 128, 512 + (pkt + 1) * 128),
                        qlg[k][:], start=(k == 0), stop=(k == KQ - 1))
                u1 = p_f32.tile([128, 512], f32, tag="f32", name="f32")
                u2 = p_f32.tile([128, 512], f32, tag="f32", name="f32")
                rp = p_rope.tile([128, 512], bf, tag="rope", name="rope")
                rope_apply(rp[:], ps_pe, cosf_t, sinf_t, sc * 512, 512, u1, u2)
                roped.append(rp)

            # --- attention, software-pipelined over j (lookahead 2) ---
            o_ps_l, dinv_l = [], []
            for i in range(HPC):
                pkt, hp = i // 2, i % 2
                o_ps = pp_o.tile([128, 512], f32, tag="po", name="po")
                o_ps_l.append(o_ps)
                pacc = p_pacc.tile([128, 512], bf, tag="pacc", name="pacc")
                nj = 4 * sc + 4
                pts = []

                def consume(j):
                    nc.tensor.matmul(o_ps[:],
                                     V_t[j][:, i * DV:(i + 1) * DV],
                                     pts[j][:], start=(j == 0),
                                     stop=(j == nj - 1))
                    if j == 0:
                        nc.vector.tensor_copy(pacc[:], pts[j][:])
                    else:
                        nc.vector.tensor_add(pacc[:], pacc[:], pts[j][:])

                for j in range(nj):
                    s_ps = pp_s.tile([128, 512], f32, tag="ps", name="ps")
                    nc.tensor.matmul(s_ps[:],
                                     K_t[i][:, j * 128:(j + 1) * 128],
                                     qn_t[i][:], start=True, stop=False)
                    if j >= 4 * sc:
                        p = j - 4 * sc
                        nc.tensor.matmul(
                            s_ps[:], negeye_t[:],
                            steps_t[:, p * 512:(p + 1) * 512],
                            start=False, stop=False)
                    nc.tensor.matmul(
                        s_ps[:],
                        kpeg[j // 4][hp * 64:(hp + 1) * 64,
                                     (j % 4) * 128:(j % 4 + 1) * 128],
                        roped[pkt][hp * 64:(hp + 1) * 64, :],
                        start=False, stop=True)
                    pt = p_wk.tile([128, 512], bf, tag="wk", name="wk")
                    nc.scalar.activation(pt[:], s_ps[:], AF.Exp)
                    pts.append(pt)
                    if j >= 2:
                        consume(j - 2)
                for j in range(max(0, nj - 2), nj):
                    consume(j)
                d_ps = pp_sm.tile([1, 512], f32, tag="sm", name="sm")
                nc.tensor.matmul(d_ps[:], ones_col[:], pacc[:],
                                 start=True, stop=True)
                dinv = p_sml.tile([1, 512], f32, tag="sml", name="sml")
                nc.vector.reciprocal_approx_fast(dinv[:], d_ps[:])
                dinv_l.append(dinv)

            oT = []
            for i in range(HPC):
                bc_sb = p_bc.tile([128, 512], f32, tag="bc", name="bc")
                nc.gpsimd.partition_broadcast(bc_sb[:], dinv_l[i][:])
                ot = p_oT.tile([128, 512], bf, tag="oT", name="oT")
                nc.vector.tensor_mul(ot[:], o_ps_l[i][:], bc_sb[:])
                oT.append(ot)

            # prefetch next chunk's gathered q_lora during w_o
            if sc + 1 < SC:
                qlg = gather_qlg(sc + 1)

            # --- w_o partial for this seq chunk ---
            for n in range(NW):
                for sbl in range(4):
                    sb = sc * 4 + sbl
                    ps = pp_s.tile([128, 512], f32, tag="ps", name="ps")
                    for i in range(HPC):
                        nc.tensor.matmul(
                            ps[:], oT[i][:, sbl * 128:(sbl + 1) * 128],
                            wo_ap(i, n), start=(i == 0),
                            stop=(i == HPC - 1))
                    ob = p_out.tile([128, 512], bf, tag="outst", name="outst")
                    nc.scalar.activation(ob[:], ps[:], AF.Copy)
                    nc.sync.dma_start(
                        out_d[sb * 128:(sb + 1) * 128, n * 512:(n + 1) * 512],
                        ob[:])

    nc.compile()
    return nc


def _prep_inputs(inputs):
    """Host-side sharding + weight folding. Returns in_maps (list of 8 dicts)."""
    BF = _bf16()

    hs = np.asarray(inputs['hidden_states'], np.float32)
    pos = np.asarray(inputs['positions'])
    w_qa = np.asarray(inputs['w_qa'], np.float32)
    q_a_ln_w = np.asarray(inputs['q_a_ln_w'], np.float32)
    w_qb = np.asarray(inputs['w_qb'], np.float32)
    w_kva = np.asarray(inputs['w_kva'], np.float32)
    kv_a_ln_w = np.asarray(inputs['kv_a_ln_w'], np.float32)
    kc = np.asarray(inputs['kc'], np.float32)
    vc = np.asarray(inputs['vc'], np.float32)
    w_o = np.asarray(inputs['w_o'], np.float32)

    perm = np.concatenate([np.arange(0, DR, 2), np.arange(1, DR, 2)])
    inv_freq = 1.0 / (ROPE_BASE ** (np.arange(0, DR, 2, dtype=np.float64) / DR))
    freqs = pos.astype(np.float64)[None, :] * inv_freq[:, None]     # [32, S]
    cosT = np.cos(freqs).astype(np.float32)
    sinT = np.sin(freqs).astype(np.float32)
    cos128 = np.tile(cosT, (4, 1)).astype(BF)                        # [128, S]
    sin128 = np.tile(sinT, (4, 1)).astype(np.float32)
    sgn = np.where((np.arange(128) % 64) < 32, -1.0, 1.0)[:, None]
    sinsg128 = (sin128 * sgn).astype(BF)

    scale = DQ ** -0.5
    w_qb_eff = ((w_qb * q_a_ln_w[:, None]) * scale).reshape(QLR, H, DQ)

    w_pe = w_kva[:, KVLR:][:, perm]
    wkvpe = np.concatenate([w_kva[:, :KVLR], w_pe, w_pe], 1).astype(BF)

    kc_f = kc * kv_a_ln_w[None, None, :]
    vc_f = vc * kv_a_ln_w[None, :, None]

    # step tables: steps[r, p*512+q] = 1 if p*128+r > q else 0
    steps = np.zeros((128, 4 * 512), np.float32)
    rr = np.arange(128)[:, None]
    qq = np.arange(512)[None, :]
    for p in range(4):
        steps[:, p * 512:(p + 1) * 512] = (p * 128 + rr > qq)
    steps_b = steps.astype(BF)
    negeye = (-1e30 * np.eye(128, dtype=np.float32)).astype(BF)

    wqa_b = w_qa.astype(BF)

    K2 = (HID // 128) // 2
    NW = HID // 512

    in_maps = []
    for core in range(NC_N):
        rows = slice(core * SL, (core + 1) * SL)
        h0 = core * HPC

        hsT = np.ascontiguousarray(hs[rows].T)                   # [HID, SL]
        hsT2 = hsT.reshape(K2, 2, 128, SL).transpose(2, 0, 1, 3) \
            .reshape(128, K2 * 2 * SL)

        wqb_all = np.empty((QLR, 768), np.float32)
        for i in range(HPC):
            wqb_all[:, i * 128:(i + 1) * 128] = w_qb_eff[:, h0 + i, :DN]
        for pkt in range(2):
            a, b = h0 + 2 * pkt, h0 + 2 * pkt + 1
            pe_a = w_qb_eff[:, a, DN:][:, perm]
            pe_b = w_qb_eff[:, b, DN:][:, perm]
            wqb_all[:, 512 + pkt * 128:512 + pkt * 128 + 64] = pe_a
            wqb_all[:, 512 + pkt * 128 + 64:512 + (pkt + 1) * 128] = pe_b
        wqb2 = wqb_all.reshape(12, 128, 768).transpose(1, 0, 2) \
            .reshape(128, 12 * 768)

        kct = np.stack([kc_f[h0 + i].T[c * 128:(c + 1) * 128]
                        for i in range(HPC) for c in range(KVLR // 128)])
        kct2 = kct.transpose(1, 0, 2).reshape(128, -1)           # [128, 2048]

        vcp = np.concatenate([vc_f[h0 + i] for i in range(HPC)], 1)
        vcp2 = vcp.reshape(KVLR // 128, 128, HPC * DV) \
            .transpose(1, 0, 2).reshape(128, -1)                 # [128, 2048]

        wo_sh = w_o[h0 * DV:(h0 + HPC) * DV, :]                  # [512, HID]
        wo2 = wo_sh.reshape(HPC, 128, NW, 512).transpose(0, 2, 1, 3) \
            .reshape(HPC * NW, 128, 512).transpose(1, 0, 2) \
            .reshape(128, -1)                                    # [128, 20480]

        in_maps.append({
            "hsT2": hsT2.astype(BF),
            "wqa": wqa_b,
            "wkvpe": wkvpe,
            "cosl": np.ascontiguousarray(cos128[:, rows]),
            "sinlsg": np.ascontiguousarray(sinsg128[:, rows]),
            "cosf": cos128,
            "sinfsg": sinsg128,
            "wqb2": wqb2.astype(BF),
            "kct2": kct2.astype(BF),
            "vcp2": vcp2.astype(BF),
            "wo2": wo2.astype(BF),
            "steps": steps_b,
            "negeye": negeye,
        })
    return in_maps


def _get_program():
    if "nc" not in _CACHE:
        _CACHE["nc"] = _build_program()
    return _CACHE["nc"]


def run(inputs, trace=False, trace_kwargs=None):
    """Build (cached), run on 8 cores, return (output, BassKernelResults)."""
    from concourse.bass_utils import run_bass_kernel_spmd

    nc = _get_program()
    in_maps = _prep_inputs(inputs)
    res = run_bass_kernel_spmd(nc, in_maps, list(range(NC_N)),
                               trace=trace, **(trace_kwargs or {}))
    out = np.zeros((S, HID), np.float32)
    for r in res.results:
        out += np.asarray(r["out_partial"], dtype=np.float32)
    return out, res


def kernel(**inputs) -> np.ndarray:
    out, _ = run(inputs, trace=False)
    return out


# revision 23
# speedup vs baseline: 1.4801x; 1.0931x over previous
"""Trainium2 Bass kernel for a DeepseekV2 decoder-layer attention block
(MLA prefill, fp32 reference) distributed across 8 NeuronCores.

Strategy (single NEFF, SPMD on 8 cores):
  - Sequence-shard the shared projections: each core computes ckv / k_pe
    (RMS-normed / roped) then q_lora for its 256 rows of the sequence, in
    transposed layout; two on-device AllGathers replicate them. ckv goes
    first so its AllGather flies under the q_lora GEMM; the q_lora
    AllGather flies under the K/V expansion.
  - Head-shard the rest (4 heads per core): q_b projection + RoPE, kc/vc
    expansion, causal attention (scores computed transposed so the attn@v
    matmul needs no transposes), and a row-shard of w_o.
  - Each core emits a partial [S, HID] bf16 output; the host sums the 8
    partials (the output all-reduce) to produce the full result.

Perf structure (v3):
  - All weight/activation streams are host-packed into [128, N] layouts so
    every DMA is one large transfer, spread round-robin across the
    sync/scalar/gpsimd/vector DGE rings (the per-DMA ~600ns issue cost made
    the v2 front end ring-bound).
  - Attention is software-pipelined (lookahead 2); the causal mask is
    applied as a third matmul into the score PSUM group (-1e30 * step),
    so the exp -> attn@v chain has no vector-engine hop.
  - Softmax denominators: bf16 P-tile running sum on the vector engine,
    one ones-matmul per (head, chunk), reciprocal_approx_fast.
  - RoPE rotation via partition-offset vector ops with sign-folded sin.
  - w_o resident; output staged bf16 through the scalar engine.
"""

import numpy as np

S, HID, H = 2048, 5120, 32
QLR, KVLR = 1536, 512
DN, DR, DV = 128, 64, 128
DQ = DN + DR
NC_N = 8
HPC = H // NC_N          # heads per core
SL = S // NC_N           # sequence rows per core (front end)
ROPE_BASE, EPS = 10000.0, 1e-6

_CACHE = {}


def _bf16():
    import ml_dtypes
    return np.dtype(ml_dtypes.bfloat16)


def _build_program():
    import concourse.bass as bass
    import concourse.tile as tile
    from concourse import bacc, mybir
    from contextlib import ExitStack

    f32 = mybir.dt.float32
    bf = mybir.dt.bfloat16
    AF = mybir.ActivationFunctionType

    nc = bacc.Bacc("TRN2", target_bir_lowering=False, debug=False,
                   num_devices=NC_N)

    def din(name, shape, dt=bf):
        return nc.dram_tensor(name, list(shape), dt, kind="ExternalInput").ap()

    KH = HID // 128       # 40 k-chunks of the model dim
    K2 = KH // 2          # 20 double-chunks
    KQ = QLR // 128       # 12 chunks of the q-lora dim
    KC = KVLR // 128      # 4 chunks of the kv-lora dim
    SC = S // 512         # 4 sequence chunks of 512
    SB = S // 128         # 16 sequence blocks of 128
    NW = HID // 512       # 10 w_o column chunks

    hsT2_d = din("hsT2", (128, K2 * 2 * SL))    # [p, k2*512+half*256+c]
    wqa_d = din("wqa", (HID, QLR))
    wkvpe2_d = din("wkvpe2", (128, (HID // 256) * 1280))  # packed pairs
    cosl_d = din("cosl", (128, SL))
    sinlsg_d = din("sinlsg", (128, SL))         # sign-folded sin
    cosf_d = din("cosf", (128, S))
    sinfsg_d = din("sinfsg", (128, S))
    wqb2_d = din("wqb2", (128, KQ * 768))       # [p, k*768 + col]
    kct2_d = din("kct2", (128, HPC * KC * 128))  # [p, (i*4+c)*128 + d]
    vcp2_d = din("vcp2", (128, KC * HPC * DV))  # [p, c*512 + col]
    wo2_d = din("wo2", (128, HPC * NW * 512))   # [p, (i*NW+n)*512 + col]
    steps_d = din("steps", (128, 4 * 512))      # [r, p*512+q] = [p*128+r > q]
    negeye_d = din("negeye", (128, 128))        # -1e30 * I
    out_d = nc.dram_tensor("out_partial", [S, HID], bf,
                           kind="ExternalOutput").ap()

    cc1_in = nc.dram_tensor("cc1_in", [KVLR + 128, SL], bf).ap()
    cc1_out = nc.dram_tensor("cc1_out", [NC_N * (KVLR + 128), SL], bf,
                             addr_space="Shared").ap()
    cc2_in = nc.dram_tensor("cc2_in", [QLR, SL], bf).ap()
    cc2_out = nc.dram_tensor("cc2_out", [NC_N * QLR, SL], bf,
                             addr_space="Shared").ap()

    with tile.TileContext(nc) as tc, ExitStack() as ctx:
        def pool(name, bufs):
            return ctx.enter_context(tc.tile_pool(name=name, bufs=bufs))

        p_hs = pool("hs", 3)
        p_w = pool("wstr", 3)
        p_raw = pool("raw", 12)
        p_sq = pool("sqt", 2)
        p_scn = pool("scn", 2)
        p_sml = pool("sml", 4)
        p_one = pool("ones", 2)
        p_cs = pool("cs", 2)
        p_csl = pool("csl", 2)
        p_bc = pool("bc", 2)
        p_kc = pool("kc", 1)
        p_vc = pool("vc", 1)
        p_wqb = pool("wqb", 2)
        p_wo = pool("wo", 5)
        p_msk = pool("msk", 1)
        p_kpeg = pool("kpeg", 4)
        p_K = pool("Kt", 4)
        p_V = pool("Vt", 16)
        p_qn = pool("Qn", 4)
        p_rope = pool("rope", 2)
        p_f32 = pool("fr32", 2)
        p_ckvg = pool("ckvg", 16)
        p_wk = pool("wk", 14)       # shared ring: qlg -> P tiles
        p_pacc = pool("pacc", 2)
        p_oT = pool("oT", 4)
        p_out = pool("outst", 3)

        pp_o = ctx.enter_context(
            tc.tile_pool(name="ppo", bufs=4, space="PSUM"))
        pp_s = ctx.enter_context(
            tc.tile_pool(name="pps", bufs=3, space="PSUM"))
        pp_sm = ctx.enter_context(
            tc.tile_pool(name="psm", bufs=1, space="PSUM"))

        ones_col = p_one.tile([128, 1], bf)       # lhsT for column sums
        nc.vector.memset(ones_col[:], 1.0)
        eps_t = p_one.tile([1, 1], f32, tag="eps", name="eps")
        nc.vector.memset(eps_t[:], EPS)

        def rope_apply(dst, src_ps, cos_t, sin_t, c0, width, u1, u2):
            """dst(bf16) = src*cos + rot(src)*sin_signed, via partition-offset
            muls. src_ps is a [128, width] f32 PSUM pack of 64-dim halves."""
            nc.vector.tensor_mul(u1[:, :width], src_ps[:],
                                 cos_t[:, c0:c0 + width])
            for blk in (0, 64):
                nc.vector.tensor_mul(
                    u2[blk:blk + 32, :width],
                    src_ps[blk + 32:blk + 64, :],
                    sin_t[blk:blk + 32, c0:c0 + width])
                nc.vector.tensor_mul(
                    u2[blk + 32:blk + 64, :width],
                    src_ps[blk:blk + 32, :],
                    sin_t[blk + 32:blk + 64, c0:c0 + width])
            nc.vector.tensor_add(dst, u1[:, :width], u2[:, :width])

        # ---------------- FRONT 1: ckv + k_pe pass (then AG1) --------------
        ckv_ps = [pp_o.tile([128, SL], f32, tag="po", name="po")
                  for _ in range(KC)]
        pe_ps = pp_s.tile([128, SL], f32, tag="ps", name="ps")
        for k2 in range(K2):
            hs2 = p_hs.tile([128, 2 * SL], bf, tag="hs", name="hs")
            nc.sync.dma_start(hs2[:], hsT2_d[:, k2 * 512:(k2 + 1) * 512])
            wv = p_w.tile([128, 1280], bf, tag="wkv", name="wkv", bufs=3)
            nc.scalar.dma_start(wv[:], wkvpe2_d[:, k2 * 1280:(k2 + 1) * 1280])
            for kk in range(2):
                k = 2 * k2 + kk
                hh = hs2[:, kk * SL:(kk + 1) * SL]
                wk0 = kk * 640
                for c in range(KC):
                    nc.tensor.matmul(ckv_ps[c][:],
                                     wv[:, wk0 + c * 128:wk0 + (c + 1) * 128],
                                     hh, start=(k == 0), stop=(k == KH - 1))
                nc.tensor.matmul(pe_ps[:], wv[:, wk0 + KVLR:wk0 + KVLR + 128],
                                 hh,
                                 start=(k == 0), stop=(k == KH - 1))
        ssq_kv = pp_sm.tile([1, SL], f32, tag="sm", name="sm")
        raw_kv = []
        for c in range(KC):
            r = p_raw.tile([128, SL], bf, tag="raw", name="raw")
            nc.scalar.activation(r[:], ckv_ps[c][:], AF.Copy)
            raw_kv.append(r)
            sq = p_sq.tile([128, SL], bf, tag="sq", name="sq")
            nc.scalar.activation(sq[:], ckv_ps[c][:], AF.Square)
            nc.tensor.matmul(ssq_kv[:], ones_col[:], sq[:],
                             start=(c == 0), stop=(c == KC - 1))
        t_kv = p_sml.tile([1, SL], f32, tag="sml", name="sml")
        nc.scalar.activation(t_kv[:], ssq_kv[:], AF.Sqrt,
                             bias=eps_t[:], scale=1.0 / KVLR)
        s_kv = p_sml.tile([1, SL], f32, tag="sml", name="sml")
        nc.vector.reciprocal(s_kv[:], t_kv[:])
        bkv_sb = p_bc.tile([128, 512], f32, tag="bc", name="bc")
        nc.gpsimd.partition_broadcast(bkv_sb[:, :SL], s_kv[:])
        for c in range(KC):
            cn = p_scn.tile([128, SL], bf, tag="scn", name="scn")
            nc.vector.tensor_mul(cn[:], raw_kv[c][:], bkv_sb[:, :SL])
            nc.scalar.dma_start(cc1_in[c * 128:(c + 1) * 128, :], cn[:])
        # rope k_pe (partition-offset rotation, sign folded into sin table)
        cosl_t = p_csl.tile([128, SL], bf, tag="csl", name="csl")
        sinl_t = p_csl.tile([128, SL], bf, tag="csl", name="csl")
        nc.sync.dma_start(cosl_t[:], cosl_d[:, :])
        nc.sync.dma_start(sinl_t[:], sinlsg_d[:, :])
        u1f = p_f32.tile([128, 512], f32, tag="f32", name="f32")
        u2f = p_f32.tile([128, 512], f32, tag="f32", name="f32")
        kpe_n = p_scn.tile([128, SL], bf, tag="scn", name="scn")
        rope_apply(kpe_n[:], pe_ps, cosl_t, sinl_t, 0, SL, u1f, u2f)
        nc.scalar.dma_start(cc1_in[KVLR:KVLR + 128, :], kpe_n[:])

        nc.gpsimd.collective_compute(
            "AllGather", mybir.AluOpType.bypass,
            ins=[cc1_in[:]], outs=[cc1_out[:]],
            replica_groups=[list(range(NC_N))],
        )

        # ---------------- FRONT 2: q_lora pass (then AG2) ------------------
        # resident back-end weights stream on the scalar/gpsimd rings
        kct_t = p_kc.tile([128, HPC * KC * 128], bf, tag="kc", name="kc")
        nc.scalar.dma_start(kct_t[:], kct2_d[:, :])
        vcp_t = p_vc.tile([128, KC * HPC * DV], bf, tag="vc", name="vc")
        nc.gpsimd.dma_start(vcp_t[:], vcp2_d[:, :])
        wqb_t = []
        for h in range(2):
            t = p_wqb.tile([128, 6 * 768], bf, tag="wqb", name="wqb")
            eng = nc.scalar if h == 0 else nc.gpsimd
            eng.dma_start(t[:], wqb2_d[:, h * 6 * 768:(h + 1) * 6 * 768])
            wqb_t.append(t)
        cosf_t = p_cs.tile([128, S], bf, tag="cs", name="cs")
        sinf_t = p_cs.tile([128, S], bf, tag="cs", name="cs")
        nc.scalar.dma_start(cosf_t[:], cosf_d[:, :])
        nc.gpsimd.dma_start(sinf_t[:], sinfsg_d[:, :])
        steps_t = p_msk.tile([128, 4 * 512], bf, tag="msk", name="msk")
        nc.scalar.dma_start(steps_t[:], steps_d[:, :])
        negeye_t = p_msk.tile([128, 128], bf, tag="ne", name="ne")
        nc.gpsimd.dma_start(negeye_t[:], negeye_d[:, :])

        def wqb_ap(k, c0, c1):
            return wqb_t[k // 6][:, (k % 6) * 768 + c0:(k % 6) * 768 + c1]

        ssq_q = pp_sm.tile([1, SL], f32, tag="sm", name="sm")
        raw_q = []
        for g in range(2):
            ql_ps = ([pp_o.tile([128, SL], f32, tag="po", name="po")
                      for _ in range(4)] +
                     [pp_s.tile([128, SL], f32, tag="ps", name="ps")
                      for _ in range(2)])
            for k2 in range(K2):
                hs2 = p_hs.tile([128, 2 * SL], bf, tag="hs", name="hs")
                heng = nc.sync if k2 % 2 == 0 else nc.scalar
                heng.dma_start(hs2[:], hsT2_d[:, k2 * 512:(k2 + 1) * 512])
                wq = []
                for kk in range(2):
                    k = 2 * k2 + kk
                    w = p_w.tile([128, QLR // 2], bf, tag="wqa", name="wqa",
                                 bufs=6)
                    eng = nc.sync if kk == 0 else nc.scalar
                    eng.dma_start(
                        w[:], wqa_d[k * 128:(k + 1) * 128,
                                    g * (QLR // 2):(g + 1) * (QLR // 2)])
                    wq.append(w)
                for kk in range(2):
                    k = 2 * k2 + kk
                    hh = hs2[:, kk * SL:(kk + 1) * SL]
                    for mi in range(6):
                        nc.tensor.matmul(ql_ps[mi][:],
                                         wq[kk][:, mi * 128:(mi + 1) * 128],
                                         hh, start=(k == 0),
                                         stop=(k == KH - 1))
            for mi in range(6):
                m = g * 6 + mi
                r = p_raw.tile([128, SL], bf, tag="raw", name="raw")
                nc.scalar.activation(r[:], ql_ps[mi][:], AF.Copy)
                raw_q.append(r)
                sq = p_sq.tile([128, SL], bf, tag="sq", name="sq")
                nc.scalar.activation(sq[:], ql_ps[mi][:], AF.Square)
                nc.tensor.matmul(ssq_q[:], ones_col[:], sq[:],
                                 start=(m == 0), stop=(m == KQ - 1))
        t_q = p_sml.tile([1, SL], f32, tag="sml", name="sml")
        nc.scalar.activation(t_q[:], ssq_q[:], AF.Sqrt,
                             bias=eps_t[:], scale=1.0 / QLR)
        s_q = p_sml.tile([1, SL], f32, tag="sml", name="sml")
        nc.vector.reciprocal(s_q[:], t_q[:])
        bq_sb = p_bc.tile([128, 512], f32, tag="bc", name="bc")
        nc.gpsimd.partition_broadcast(bq_sb[:, :SL], s_q[:])
        for m in range(KQ):
            qn = p_scn.tile([128, SL], bf, tag="scn", name="scn")
            nc.vector.tensor_mul(qn[:], raw_q[m][:], bq_sb[:, :SL])
            nc.scalar.dma_start(cc2_in[m * 128:(m + 1) * 128, :], qn[:])

        nc.gpsimd.collective_compute(
            "AllGather", mybir.AluOpType.bypass,
            ins=[cc2_in[:]], outs=[cc2_out[:]],
            replica_groups=[list(range(NC_N))],
        )

        # ---------------- BACK: K/V expansion (needs AG1 only) -------------
        RPC = 512 // SL     # AG rank-blocks per 512-wide seq chunk
        g_engs = [nc.scalar, nc.gpsimd, nc.sync]
        g_rr = [0]

        def rr_eng():
            e = g_engs[g_rr[0] % 3]
            g_rr[0] += 1
            return e

        ckvg = {}
        for c in range(KC):
            for sc in range(SC):
                t = p_ckvg.tile([128, 512], bf, tag="ckvg", name="ckvg")
                for half in range(RPC):
                    r = RPC * sc + half
                    rr_eng().dma_start(
                        t[:, half * SL:(half + 1) * SL],
                        cc1_out[r * (KVLR + 128) + c * 128:
                                r * (KVLR + 128) + (c + 1) * 128, :])
                ckvg[(c, sc)] = t
        kpeg = {}
        for sc in range(SC):
            t = p_kpeg.tile([128, 512], bf, tag="kpeg", name="kpeg")
            for half in range(RPC):
                r = RPC * sc + half
                rr_eng().dma_start(
                    t[:, half * SL:(half + 1) * SL],
                    cc1_out[r * (KVLR + 128) + KVLR:
                            r * (KVLR + 128) + KVLR + 128, :])
            kpeg[sc] = t
        # w_o resident tiles: streamed after the gathers
        wo_t = []
        for h in range(5):
            t = p_wo.tile([128, 8 * 512], bf, tag="wo", name="wo")
            eng = g_engs[h % 2]
            eng.dma_start(t[:], wo2_d[:, h * 8 * 512:(h + 1) * 8 * 512])
            wo_t.append(t)

        def wo_ap(i, n):
            b = i * NW + n
            return wo_t[b // 8][:, (b % 8) * 512:(b % 8 + 1) * 512]

        # K^T per head: [DN, S]
        K_t = []
        for i in range(HPC):
            kt = p_K.tile([128, S], bf, tag="K", name="K")
            K_t.append(kt)
            for sc in range(SC):
                ps = pp_s.tile([128, 512], f32, tag="ps", name="ps")
                for c in range(KC):
                    nc.tensor.matmul(
                        ps[:],
                        kct_t[:, (i * KC + c) * 128:(i * KC + c + 1) * 128],
                        ckvg[(c, sc)][:],
                        start=(c == 0), stop=(c == KC - 1))
                nc.scalar.activation(kt[:, sc * 512:(sc + 1) * 512], ps[:],
                                     AF.Copy)

        # V natural: per seq-block [128, 4*DV]
        V_t = []
        for sb in range(SB):
            ps = pp_s.tile([128, 512], f32, tag="ps", name="ps")
            for c in range(KC):
                nc.tensor.matmul(
                    ps[:],
                    ckvg[(c, sb // 4)][:, (sb % 4) * 128:(sb % 4 + 1) * 128],
                    vcp_t[:, c * 512:(c + 1) * 512],
                    start=(c == 0), stop=(c == KC - 1))
            vt = p_V.tile([128, HPC * DV], bf, tag="V", name="V")
            nc.scalar.activation(vt[:], ps[:], AF.Copy)
            V_t.append(vt)

        # ---------------- BACK: per-chunk Q proj + attention + w_o ---------
        qg_engs = [nc.gpsimd, nc.scalar]

        def gather_qlg(sc):
            qlg = []
            for k in range(KQ):
                t = p_wk.tile([128, 512], bf, tag="wk", name="wk")
                for half in range(RPC):
                    r = RPC * sc + half
                    qg_engs[(2 * k + half) % 2].dma_start(
                        t[:, half * SL:(half + 1) * SL],
                        cc2_out[r * QLR + k * 128:r * QLR + (k + 1) * 128, :])
                qlg.append(t)
            return qlg

        qlg = gather_qlg(0)
        for sc in range(SC):
            # --- Q^T nope per head + pe packs (roped) ---
            qn_t = []
            for i in range(HPC):
                ps = pp_o.tile([128, 512], f32, tag="po", name="po")
                for k in range(KQ):
                    nc.tensor.matmul(ps[:], wqb_ap(k, i * 128, (i + 1) * 128),
                                     qlg[k][:], start=(k == 0),
                                     stop=(k == KQ - 1))
                qt = p_qn.tile([128, 512], bf, tag="Qn", name="Qn")
                nc.scalar.activation(qt[:], ps[:], AF.Copy)
                qn_t.append(qt)
            roped = []
            for pkt in range(2):
                ps_pe = pp_s.tile([128, 512], f32, tag="ps", name="ps")
                for k in range(KQ):
                    nc.tensor.matmul(
                        ps_pe[:],
                        wqb_ap(k, 512 + pkt * 128, 512 + (pkt + 1) * 128),
                        qlg[k][:], start=(k == 0), stop=(k == KQ - 1))
                u1 = p_f32.tile([128, 512], f32, tag="f32", name="f32")
                u2 = p_f32.tile([128, 512], f32, tag="f32", name="f32")
                rp = p_rope.tile([128, 512], bf, tag="rope", name="rope")
                rope_apply(rp[:], ps_pe, cosf_t, sinf_t, sc * 512, 512, u1, u2)
                roped.append(rp)

            # --- attention, software-pipelined over j (lookahead 2) ---
            o_ps_l, dinv_l = [], []
            for i in range(HPC):
                pkt, hp = i // 2, i % 2
                o_ps = pp_o.tile([128, 512], f32, tag="po", name="po")
                o_ps_l.append(o_ps)
                pacc = p_pacc.tile([128, 512], bf, tag="pacc", name="pacc")
                nj = 4 * sc + 4
                pts = []

                def consume(j):
                    nc.tensor.matmul(o_ps[:],
                                     V_t[j][:, i * DV:(i + 1) * DV],
                                     pts[j][:], start=(j == 0),
                                     stop=(j == nj - 1))
                    if j == 0:
                        nc.vector.tensor_copy(pacc[:], pts[j][:])
                    else:
                        nc.vector.tensor_add(pacc[:], pacc[:], pts[j][:])

                for j in range(nj):
                    s_ps = pp_s.tile([128, 512], f32, tag="ps", name="ps")
                    nc.tensor.matmul(s_ps[:],
                                     K_t[i][:, j * 128:(j + 1) * 128],
                                     qn_t[i][:], start=True, stop=False)
                    if j >= 4 * sc:
                        p = j - 4 * sc
                        nc.tensor.matmul(
                            s_ps[:], negeye_t[:],
                            steps_t[:, p * 512:(p + 1) * 512],
                            start=False, stop=False)
                    nc.tensor.matmul(
                        s_ps[:],
                        kpeg[j // 4][hp * 64:(hp + 1) * 64,
                                     (j % 4) * 128:(j % 4 + 1) * 128],
                        roped[pkt][hp * 64:(hp + 1) * 64, :],
                        start=False, stop=True)
                    pt = p_wk.tile([128, 512], bf, tag="wk", name="wk")
                    nc.scalar.activation(pt[:], s_ps[:], AF.Exp)
                    pts.append(pt)
                    if j >= 2:
                        consume(j - 2)
                for j in range(max(0, nj - 2), nj):
                    consume(j)
                d_ps = pp_sm.tile([1, 512], f32, tag="sm", name="sm")
                nc.tensor.matmul(d_ps[:], ones_col[:], pacc[:],
                                 start=True, stop=True)
                dinv = p_sml.tile([1, 512], f32, tag="sml", name="sml")
                nc.vector.reciprocal_approx_fast(dinv[:], d_ps[:])
                dinv_l.append(dinv)

            oT = []
            for i in range(HPC):
                bc_sb = p_bc.tile([128, 512], f32, tag="bc", name="bc")
                nc.gpsimd.partition_broadcast(bc_sb[:], dinv_l[i][:])
                ot = p_oT.tile([128, 512], bf, tag="oT", name="oT")
                nc.vector.tensor_mul(ot[:], o_ps_l[i][:], bc_sb[:])
                oT.append(ot)

            # prefetch next chunk's gathered q_lora during w_o
            if sc + 1 < SC:
                qlg = gather_qlg(sc + 1)

            # --- w_o partial for this seq chunk ---
            # sbl outer / paired n inner: two [128,512] psum groups stage
            # into one [128,1024] tile and ship with a single DMA
            for sbl in range(4):
                sb = sc * 4 + sbl
                for n2 in range(NW // 2):
                    ob = p_out.tile([128, 1024], bf, tag="outst",
                                    name="outst")
                    for half in range(2):
                        n = 2 * n2 + half
                        ps = pp_s.tile([128, 512], f32, tag="ps", name="ps")
                        for i in range(HPC):
                            nc.tensor.matmul(
                                ps[:], oT[i][:, sbl * 128:(sbl + 1) * 128],
                                wo_ap(i, n), start=(i == 0),
                                stop=(i == HPC - 1))
                        ceng = nc.scalar if half == 0 else nc.vector
                        if half == 0:
                            ceng.activation(ob[:, :512], ps[:], AF.Copy)
                        else:
                            ceng.tensor_copy(ob[:, 512:], ps[:])
                    nc.sync.dma_start(
                        out_d[sb * 128:(sb + 1) * 128,
                              n2 * 1024:(n2 + 1) * 1024], ob[:])

    nc.compile()
    return nc


def _prep_inputs(inputs):
    """Host-side sharding + weight folding. Returns in_maps (list of 8 dicts)."""
    BF = _bf16()

    hs = np.asarray(inputs['hidden_states'], np.float32)
    pos = np.asarray(inputs['positions'])
    w_qa = np.asarray(inputs['w_qa'], np.float32)
    q_a_ln_w = np.asarray(inputs['q_a_ln_w'], np.float32)
    w_qb = np.asarray(inputs['w_qb'], np.float32)
    w_kva = np.asarray(inputs['w_kva'], np.float32)
    kv_a_ln_w = np.asarray(inputs['kv_a_ln_w'], np.float32)
    kc = np.asarray(inputs['kc'], np.float32)
    vc = np.asarray(inputs['vc'], np.float32)
    w_o = np.asarray(inputs['w_o'], np.float32)

    perm = np.concatenate([np.arange(0, DR, 2), np.arange(1, DR, 2)])
    inv_freq = 1.0 / (ROPE_BASE ** (np.arange(0, DR, 2, dtype=np.float64) / DR))
    freqs = pos.astype(np.float64)[None, :] * inv_freq[:, None]     # [32, S]
    cosT = np.cos(freqs).astype(np.float32)
    sinT = np.sin(freqs).astype(np.float32)
    cos128 = np.tile(cosT, (4, 1)).astype(BF)                        # [128, S]
    sin128 = np.tile(sinT, (4, 1)).astype(np.float32)
    sgn = np.where((np.arange(128) % 64) < 32, -1.0, 1.0)[:, None]
    sinsg128 = (sin128 * sgn).astype(BF)

    scale = DQ ** -0.5
    w_qb_eff = ((w_qb * q_a_ln_w[:, None]) * scale).reshape(QLR, H, DQ)

    w_pe = w_kva[:, KVLR:][:, perm]
    wkvpe = np.concatenate([w_kva[:, :KVLR], w_pe, w_pe], 1)   # [HID, 640]
    K2h = (HID // 128) // 2
    wkvpe2 = wkvpe.reshape(K2h, 2, 128, 640).transpose(2, 0, 1, 3) \
        .reshape(128, K2h * 1280).astype(BF)

    kc_f = kc * kv_a_ln_w[None, None, :]
    vc_f = vc * kv_a_ln_w[None, :, None]

    # step tables: steps[r, p*512+q] = 1 if p*128+r > q else 0
    steps = np.zeros((128, 4 * 512), np.float32)
    rr = np.arange(128)[:, None]
    qq = np.arange(512)[None, :]
    for p in range(4):
        steps[:, p * 512:(p + 1) * 512] = (p * 128 + rr > qq)
    steps_b = steps.astype(BF)
    negeye = (-1e30 * np.eye(128, dtype=np.float32)).astype(BF)

    wqa_b = w_qa.astype(BF)

    K2 = (HID // 128) // 2
    NW = HID // 512

    in_maps = []
    for core in range(NC_N):
        rows = slice(core * SL, (core + 1) * SL)
        h0 = core * HPC

        hsT = np.ascontiguousarray(hs[rows].T)                   # [HID, SL]
        hsT2 = hsT.reshape(K2, 2, 128, SL).transpose(2, 0, 1, 3) \
            .reshape(128, K2 * 2 * SL)

        wqb_all = np.empty((QLR, 768), np.float32)
        for i in range(HPC):
            wqb_all[:, i * 128:(i + 1) * 128] = w_qb_eff[:, h0 + i, :DN]
        for pkt in range(2):
            a, b = h0 + 2 * pkt, h0 + 2 * pkt + 1
            pe_a = w_qb_eff[:, a, DN:][:, perm]
            pe_b = w_qb_eff[:, b, DN:][:, perm]
            wqb_all[:, 512 + pkt * 128:512 + pkt * 128 + 64] = pe_a
            wqb_all[:, 512 + pkt * 128 + 64:512 + (pkt + 1) * 128] = pe_b
        wqb2 = wqb_all.reshape(12, 128, 768).transpose(1, 0, 2) \
            .reshape(128, 12 * 768)

        kct = np.stack([kc_f[h0 + i].T[c * 128:(c + 1) * 128]
                        for i in range(HPC) for c in range(KVLR // 128)])
        kct2 = kct.transpose(1, 0, 2).reshape(128, -1)           # [128, 2048]

        vcp = np.concatenate([vc_f[h0 + i] for i in range(HPC)], 1)
        vcp2 = vcp.reshape(KVLR // 128, 128, HPC * DV) \
            .transpose(1, 0, 2).reshape(128, -1)                 # [128, 2048]

        wo_sh = w_o[h0 * DV:(h0 + HPC) * DV, :]                  # [512, HID]
        wo2 = wo_sh.reshape(HPC, 128, NW, 512).transpose(0, 2, 1, 3) \
            .reshape(HPC * NW, 128, 512).transpose(1, 0, 2) \
            .reshape(128, -1)                                    # [128, 20480]

        in_maps.append({
            "hsT2": hsT2.astype(BF),
            "wqa": wqa_b,
            "wkvpe2": wkvpe2,
            "cosl": np.ascontiguousarray(cos128[:, rows]),
            "sinlsg": np.ascontiguousarray(sinsg128[:, rows]),
            "cosf": cos128,
            "sinfsg": sinsg128,
            "wqb2": wqb2.astype(BF),
            "kct2": kct2.astype(BF),
            "vcp2": vcp2.astype(BF),
            "wo2": wo2.astype(BF),
            "steps": steps_b,
            "negeye": negeye,
        })
    return in_maps


def _get_program():
    if "nc" not in _CACHE:
        _CACHE["nc"] = _build_program()
    return _CACHE["nc"]


def run(inputs, trace=False, trace_kwargs=None):
    """Build (cached), run on 8 cores, return (output, BassKernelResults)."""
    from concourse.bass_utils import run_bass_kernel_spmd

    nc = _get_program()
    in_maps = _prep_inputs(inputs)
    res = run_bass_kernel_spmd(nc, in_maps, list(range(NC_N)),
                               trace=trace, **(trace_kwargs or {}))
    out = np.zeros((S, HID), np.float32)
    for r in res.results:
        out += np.asarray(r["out_partial"], dtype=np.float32)
    return out, res


def kernel(**inputs) -> np.ndarray:
    out, _ = run(inputs, trace=False)
    return out
